# revision 18
# baseline (speedup 1.0000x reference)
"""Chamfer L2 loss (nn_ChamferL2Loss) Trainium2 Bass kernel.

Strategy: 8 NeuronCores, core c handles batch b=c//2 and target-half h=c%2.
Each core computes row-mins of the [7000 x 3500] squared-distance matrix for
its half via K=4 matmuls (coords + fused |t|^2 + column-mask row), DVE
reduce-min from PSUM, then an AllReduce(min) within core pairs merges halves.
The kth-value threshold (jnp.sort + take in the reference) is found with an
exact bit-pattern multi-way bisection (high-23/low-8 bit split keeps all DVE
integer arithmetic within fp32-exact range).  Final per-batch losses are
averaged with an 8-core AllReduce(add).
"""

import numpy as np

import concourse.bass as bass
import concourse.tile as tile
import concourse.mybir as mybir
from concourse.alu_op_type import AluOpType
from concourse.vector_clock import ScopedClock
from concourse.bass_utils import run_bass_kernel_spmd

f32 = mybir.dt.float32
bf16 = mybir.dt.bfloat16
i32 = mybir.dt.int32
AX = mybir.AxisListType
AF = mybir.ActivationFunctionType

B = 4
N = 7000          # points per cloud
NI = 7040         # padded rows (55 * 128)
AI = 55           # NI / 128
MH = 3500         # targets per core (half)
NJ = 3584         # padded cols (28 * 128 = 8 * 448)
AJ = 28           # NJ / 128
JT = 448          # matmul free-dim tile
BIG = np.float32(1e10)
PADV = np.float32(1e4)
MARGIN = 0.05
MIN_PTS = 500.0
HB_HI = 8388609   # 2^23 + 1: exclusive upper bound for high-23-bit patterns

N_CORES = 8


# --------------------------------------------------------------------------
# Custom DVE op: out = min(in0, in1); accum_out = min(C0, min_k out[k]).
# Consumes two tiles per instruction (both DVE read ports), halving the
# per-element cost of the row-min versus tensor_reduce.  Registered via the
# documented extension point in concourse.dve_ops (define + append to OPS).
# --------------------------------------------------------------------------
def _register_minmin():
    from concourse import dve_ops
    from concourse.dve_spec import Spec, Src0, Src1, C0, minn
    name = "TT_MIN_REDUCE_ANT"
    for o in dve_ops.OPS:
        if o.name == name:
            return o
    op = dve_ops.DveOp(
        name,
        Spec(body=minn(Src0, Src1), accum=minn, accum_init=C0,
             reference=lambda in0, in1, c0, c1, c2: np.minimum(
                 in0.astype(np.float32), in1.astype(np.float32))),
        subdim=False,
        uops_sha={"v3": "80668f319ac378ba", "v4": "23f6c1536de15f6a"},
    )
    dve_ops.OPS.append(op)
    dve_ops.CUSTOM_DVE_SPECS[name] = op.spec
    dve_ops._SUB_OPCODE_FOR_NAME[name] = max(dve_ops._SUB_OPCODE_FOR_NAME.values()) + 1
    assert dve_ops._SUB_OPCODE_FOR_NAME[name] < 0x20
    return op


MINMIN = _register_minmin()


# --------------------------------------------------------------------------
# TileContext workaround: this container's walrus build rejects instructions
# carrying more than one semaphore wait ("Too many sync wait commands").
# Split extra waits onto single-wait NOPs inserted just before the holder.
# --------------------------------------------------------------------------
def _split_multiwaits(nc, max_waits=1):
    for f in nc.m.functions:
        for bb in f.blocks:
            insts = bb.instructions
            idx = 0
            while idx < len(insts):
                inst = insts[idx]
                si = inst.sync_info
                if si is not None and len(si.on_wait) > max_waits:
                    waits = list(si.on_wait)
                    inst.sync_info = mybir.SyncInfo(
                        on_wait=waits[:max_waits], on_update=list(si.on_update))
                    for w in waits[max_waits:]:
                        nop = mybir.InstNoOp(
                            name=f"waitsplit-{nc.next_id()}", ins=[], outs=[])
                        nop.engine = inst.engine
                        nop.sync_info = mybir.SyncInfo(on_wait=[w], on_update=[])
                        nc.register_instruction(nop)
                        insts.insert(idx, nop)
                        idx += 1
                idx += 1


class TC(tile.TileContext):
    def schedule_and_allocate(self, validate_deps=False):
        r = super().schedule_and_allocate(validate_deps=validate_deps)
        _split_multiwaits(self.nc)
        return r


# --------------------------------------------------------------------------
# device program
# --------------------------------------------------------------------------
def _ptree_fold32(nc, pool, src, op):
    """Reduce [128, F] across partitions to [32, F] via 2 pairwise folds
    (engine SBUF accesses must start at 32-aligned partitions)."""
    f = src.shape[-1]
    h64 = pool.tile([64, f], f32, name=f"foldc64_{nc.next_id()}")
    nc.vector.tensor_copy(h64[:], src[64:128, :])
    t64 = pool.tile([64, f], f32, name=f"fold64_{nc.next_id()}")
    nc.vector.tensor_tensor(out=t64[:], in0=src[0:64, :], in1=h64[:], op=op)
    h32 = pool.tile([32, f], f32, name=f"foldc32_{nc.next_id()}")
    nc.vector.tensor_copy(h32[:], t64[32:64, :])
    t32 = pool.tile([32, f], f32, name=f"fold32_{nc.next_id()}")
    nc.vector.tensor_tensor(out=t32[:], in0=t64[0:32, :], in1=h32[:], op=op)
    return t32


def build_nc():
    nc = bass.Bass(num_devices=N_CORES)

    pred_pm = nc.declare_dram_parameter('pred_pm', [128, AI * 3], f32, isOutput=False)
    pred_nat = nc.declare_dram_parameter('pred_nat', [128, AI * 3], f32, isOutput=False)
    tgt_nat = nc.declare_dram_parameter('tgt_nat', [128, AI * 3], f32, isOutput=False)
    tgt_half_pm = nc.declare_dram_parameter('tgt_half_pm', [128, AJ * 3], f32, isOutput=False)
    mask_nat = nc.declare_dram_parameter('mask_nat', [128, AI], f32, isOutput=False)
    valid_nat = nc.declare_dram_parameter('valid_nat', [128, AI], f32, isOutput=False)
    alpha_in = nc.declare_dram_parameter('alpha_in', [1, 1], f32, isOutput=False)

    out_d = nc.declare_dram_parameter('out', [1, 1], f32, isOutput=True)
    dbg_d = nc.declare_dram_parameter('dbg', [128, 8], f32, isOutput=True)
    dbg2_d = nc.declare_dram_parameter('dbg2', [128, 8], f32, isOutput=True)
    dbg_diff = nc.declare_dram_parameter('dbg_diff', [128, AI], f32, isOutput=True)

    with TC(nc) as tc:
        with tc.tile_pool(name='const', bufs=1) as cp, \
             tc.tile_pool(name='work', bufs=2) as wp, \
             tc.tile_pool(name='dram', bufs=1, space='DRAM') as dp:

            # ---------- loads ----------
            # bf16 split-precision matmul, K=21:
            #   lhsT rows: P1 P1 P1 P2 P2 P3 (x3 coords) + three ones rows
            #   rhs rows:  V1 V2 V3 V1 V2 V1 (x3 coords, V=-2t) + w1 w2 w3
            # where X = X1+X2+X3 is a 3-term bf16 split and w is the 3-term
            # split of |t|^2 + (1-tsel)*BIG.  Dropped cross terms are
            # O(|p||t| 2^-26).  Rows are assembled via a DRAM staging
            # buffer (engine writes must start at 32-aligned partitions,
            # DMA round-trip through DRAM sidesteps that).
            ppm = cp.tile([128, AI * 3], f32)
            nc.sync.dma_start(ppm[:], pred_pm[:])

            pnat = cp.tile([128, AI * 3], f32)
            nc.sync.dma_start(pnat[:], pred_nat[:])
            tnat = cp.tile([128, AI * 3], f32)
            nc.sync.dma_start(tnat[:], tgt_nat[:])
            thpm = cp.tile([128, AJ * 3], f32)
            nc.sync.dma_start(thpm[:], tgt_half_pm[:])
            mnat = cp.tile([128, AI], f32)
            nc.sync.dma_start(mnat[:], mask_nat[:])
            vnat = cp.tile([128, AI], f32)
            nc.sync.dma_start(vnat[:], valid_nat[:])
            alph = cp.tile([1, 1], f32)
            nc.sync.dma_start(alph[:], alpha_in[:])

            ones = cp.tile([128, 128], f32)
            nc.vector.memset(ones[:], 1.0)

            pnat3 = pnat[:].rearrange("p (a k) -> p a k", k=3)
            tnat3 = tnat[:].rearrange("p (a k) -> p a k", k=3)
            thpm3 = thpm[:].rearrange("p (a k) -> p a k", k=3)

            ppm3 = ppm[:].rearrange("p (a k) -> p a k", k=3)

            stage_l = dp.tile([21, NI], f32)
            stage_r = dp.tile([21, NJ], f32)
            onesAI = wp.tile([128, AI], f32)
            nc.vector.memset(onesAI[:], 1.0)

            def cdu(dst, src_ap, cols, tagn):
                # dst <- f32(bf16(src)): round-trip through bf16
                tmpb = wp.tile([128, cols], bf16, name=f"cdub_{nc.next_id()}", tag=f"cdub{tagn}")
                nc.vector.tensor_copy(tmpb[:], src_ap)
                nc.vector.tensor_copy(dst[:], tmpb[:])

            def split3(src_ap, cols, tagn):
                s1 = wp.tile([128, cols], f32, name=f"s1_{nc.next_id()}", tag=f"s1{tagn}")
                s2 = wp.tile([128, cols], f32, name=f"s2_{nc.next_id()}", tag=f"s2{tagn}")
                s3 = wp.tile([128, cols], f32, name=f"s3_{nc.next_id()}", tag=f"s3{tagn}")
                r = wp.tile([128, cols], f32, name=f"r_{nc.next_id()}", tag=f"r{tagn}")
                cdu(s1, src_ap, cols, tagn)
                nc.vector.tensor_tensor(out=r[:], in0=src_ap, in1=s1[:], op=AluOpType.subtract)
                cdu(s2, r[:], cols, tagn)
                nc.vector.tensor_tensor(out=r[:], in0=r[:], in1=s2[:], op=AluOpType.subtract)
                cdu(s3, r[:], cols, tagn)
                return s1, s2, s3

            # lhsT planes: pred splits (pm layout, point = p*AI + a)
            for k in range(3):
                p1, p2, p3 = split3(ppm3[:, :, k], AI, "p")
                for row, t in ((0, p1), (3, p1), (6, p1), (9, p2), (12, p2), (15, p3)):
                    nc.sync.dma_start(stage_l[row + k:row + k + 1, :], t[:])
            for row in (18, 19, 20):
                nc.sync.dma_start(stage_l[row:row + 1, :], onesAI[:])

            # rhs coordinate planes: V = -2*t splits (pm layout, point = p*AJ + a)
            for k in range(3):
                vneg = wp.tile([128, AJ], f32, name=f"vneg_{k}", tag="vneg")
                nc.vector.tensor_scalar(out=vneg[:], in0=thpm3[:, :, k], scalar1=-2.0, scalar2=None, op0=AluOpType.mult)
                t1, t2, t3 = split3(vneg[:], AJ, "t")
                for row, t in ((0, t1), (3, t2), (6, t3), (9, t1), (12, t2), (15, t1)):
                    nc.sync.dma_start(stage_r[row + k:row + k + 1, :], t[:])

            lhsT_f = cp.tile([21, NI], f32)
            nc.sync.dma_start(lhsT_f[:], stage_l[:])
            lhsT_bf = cp.tile([21, NI], bf16)
            nc.vector.tensor_copy(lhsT_bf[:], lhsT_f[:])

            # ---------- |p|^2 per point (nat layout), |t|^2 per half target (pm layout)
            sqp = wp.tile([128, AI * 3], f32)
            nc.vector.tensor_tensor(out=sqp[:], in0=pnat[:], in1=pnat[:], op=AluOpType.mult)
            sqp3 = sqp[:].rearrange("p (a k) -> p a k", k=3)
            pp = cp.tile([128, AI], f32)
            nc.vector.tensor_tensor(out=pp[:], in0=sqp3[:, :, 0], in1=sqp3[:, :, 1], op=AluOpType.add)
            nc.vector.tensor_tensor(out=pp[:], in0=pp[:], in1=sqp3[:, :, 2], op=AluOpType.add)

            sqt = wp.tile([128, AJ * 3], f32)
            nc.vector.tensor_tensor(out=sqt[:], in0=thpm[:], in1=thpm[:], op=AluOpType.mult)
            sqt3 = sqt[:].rearrange("p (a k) -> p a k", k=3)
            ttpm = cp.tile([128, AJ], f32)
            nc.vector.tensor_tensor(out=ttpm[:], in0=sqt3[:, :, 0], in1=sqt3[:, :, 1], op=AluOpType.add)
            nc.vector.tensor_tensor(out=ttpm[:], in0=ttpm[:], in1=sqt3[:, :, 2], op=AluOpType.add)

            # ---------- bounds from pred (exact min/max over the 7000 real rows)
            # pad rows hold PADV=1e4 > any real coord, fine for max; for min
            # they lose to real values (real coords ~N(0,1), all < 1e4).
            # PADV would corrupt min only if all values padded - not the case.
            mx32 = _ptree_fold32(nc, wp, pnat[:], AluOpType.max)   # [32, 165]
            mn32 = _ptree_fold32(nc, wp, pnat[:], AluOpType.min)   # [32, 165]
            mxf = wp.tile([1, 32 * AI * 3], f32)
            mnf = wp.tile([1, 32 * AI * 3], f32)
            nc.sync.dma_start(mxf[:], mx32[:])
            nc.sync.dma_start(mnf[:], mn32[:])
            mx13 = wp.tile([1, 3], f32)
            mn13 = wp.tile([1, 3], f32)
            mx1v = mxf[:].rearrange("o (g a k) -> o k g a", k=3, a=AI)
            mn1v = mnf[:].rearrange("o (g a k) -> o k g a", k=3, a=AI)
            nc.vector.tensor_reduce(mx13[:], mx1v, axis=AX.XY, op=AluOpType.max)
            nc.vector.tensor_reduce(mn13[:], mn1v, axis=AX.XY, op=AluOpType.min)

            # lo = mn + 0.05*w ; hi = mx - 0.05*w ; w = mx - mn     (f32, as ref)
            w13 = wp.tile([1, 3], f32)
            nc.vector.tensor_tensor(out=w13[:], in0=mx13[:], in1=mn13[:], op=AluOpType.subtract)
            mw = wp.tile([1, 3], f32)
            nc.vector.tensor_scalar(out=mw[:], in0=w13[:], scalar1=float(MARGIN), scalar2=None, op0=AluOpType.mult)
            lo13 = wp.tile([1, 3], f32)
            nc.vector.tensor_tensor(out=lo13[:], in0=mn13[:], in1=mw[:], op=AluOpType.add)
            hi13 = wp.tile([1, 3], f32)
            nc.vector.tensor_tensor(out=hi13[:], in0=mx13[:], in1=mw[:], op=AluOpType.subtract)
            hl13 = wp.tile([1, 3], f32)
            nc.vector.tensor_tensor(out=hl13[:], in0=hi13[:], in1=lo13[:], op=AluOpType.subtract)
            # r_lo = (hi-lo)*bi*bs + lo ; r_hi = r_lo + (hi-lo)*bs
            bibs = wp.tile([1, 3], f32)   # bi*bs = [0.4, 0, 0]
            nc.vector.memset(bibs[:], 0.0)
            nc.vector.memset(bibs[0:1, 0:1], 0.4)
            bs13 = wp.tile([1, 3], f32)   # bs = [0.1, 1, 1]
            nc.vector.memset(bs13[:], 1.0)
            nc.vector.memset(bs13[0:1, 0:1], 0.1)
            t13 = wp.tile([1, 3], f32)
            nc.vector.tensor_tensor(out=t13[:], in0=hl13[:], in1=bibs[:], op=AluOpType.mult)
            rlo13 = wp.tile([1, 6], f32)
            nc.vector.tensor_tensor(out=rlo13[:, 0:3], in0=t13[:], in1=lo13[:], op=AluOpType.add)
            nc.vector.tensor_tensor(out=t13[:], in0=hl13[:], in1=bs13[:], op=AluOpType.mult)
            nc.vector.tensor_tensor(out=rlo13[:, 3:6], in0=rlo13[:, 0:3], in1=t13[:], op=AluOpType.add)

            # broadcast [1,6] -> [128,6] via K=1 matmul with ones
            with tc.tile_pool(name='ps_pre', bufs=1, space='PSUM') as psp:
                rl_ps = psp.tile([128, 6], f32)
                nc.tensor.matmul(rl_ps[:], lhsT=ones[0:1, :], rhs=rlo13[:], start=True, stop=True)
                rlh = cp.tile([128, 6], f32)
                nc.vector.tensor_copy(rlh[:], rl_ps[:])

                # ---------- indicators (strict > r_lo and < r_hi on all 3 dims)
                def indicator(dst, src3, acols):
                    tmp = wp.tile([128, acols], f32, name=f"indt_{nc.next_id()}", tag="indt")
                    for k in range(3):
                        nc.vector.tensor_scalar(out=(dst if k == 0 else tmp)[:, 0:acols], in0=src3[:, :, k],
                                                scalar1=rlh[:, k:k + 1], scalar2=None, op0=AluOpType.is_gt)
                        if k > 0:
                            nc.vector.tensor_tensor(out=dst[:, 0:acols], in0=dst[:, 0:acols], in1=tmp[:, 0:acols], op=AluOpType.mult)
                        nc.vector.tensor_scalar(out=tmp[:, 0:acols], in0=src3[:, :, k],
                                                scalar1=rlh[:, 3 + k:4 + k], scalar2=None, op0=AluOpType.is_lt)
                        nc.vector.tensor_tensor(out=dst[:, 0:acols], in0=dst[:, 0:acols], in1=tmp[:, 0:acols], op=AluOpType.mult)

                ip = cp.tile([128, AI], f32)
                indicator(ip, pnat3, AI)
                # pred_nat pads replicate point 0 (keeps bounds exact), so
                # mask pad rows out of the indicator explicitly
                nc.vector.tensor_tensor(out=ip[:], in0=ip[:], in1=vnat[:], op=AluOpType.mult)
                itf = wp.tile([128, AI], f32)
                indicator(itf, tnat3, AI)
                ith = cp.tile([128, AJ], f32)
                indicator(ith, thpm3, AJ)

                dbg2t = wp.tile([128, 8], f32)
                nc.vector.tensor_copy(dbg2t[:, 0:6], rlh[:])
                nc.vector.tensor_reduce(dbg2t[:, 6:7], ip[:], axis=AX.X, op=AluOpType.add)
                nc.vector.tensor_reduce(dbg2t[:, 7:8], pnat3[:, :, 0], axis=AX.X, op=AluOpType.max)
                nc.sync.dma_start(dbg2_d[:], dbg2t[:])

                # counts over full clouds (pads indicate 0)
                c2 = wp.tile([128, 2], f32)
                nc.vector.tensor_reduce(c2[:, 0:1], ip[:], axis=AX.X, op=AluOpType.add)
                nc.vector.tensor_reduce(c2[:, 1:2], itf[:], axis=AX.X, op=AluOpType.add)
                c2_ps = psp.tile([128, 2], f32)
                nc.tensor.matmul(c2_ps[:], lhsT=ones[:], rhs=c2[:], start=True, stop=True)
                c2a = cp.tile([128, 2], f32)
                nc.vector.tensor_copy(c2a[:], c2_ps[:])

                # psel = ip if n_ip >= 500 else onehot0
                flagp = cp.tile([128, 1], f32)
                nc.vector.tensor_scalar(out=flagp[:], in0=c2a[:, 0:1], scalar1=MIN_PTS, scalar2=None, op0=AluOpType.is_ge)
                invp = cp.tile([128, 1], f32)
                nc.vector.tensor_scalar(out=invp[:], in0=flagp[:], scalar1=-1.0, scalar2=None, op0=AluOpType.mult)
                nc.vector.tensor_scalar(out=invp[:], in0=invp[:], scalar1=1.0, scalar2=None, op0=AluOpType.add)
                psel = cp.tile([128, AI], f32)
                nc.vector.tensor_scalar(out=psel[:], in0=ip[:], scalar1=flagp[:], scalar2=None, op0=AluOpType.mult)
                oneh = wp.tile([128, AI], f32)
                nc.vector.memset(oneh[:], 0.0)
                nc.vector.memset(oneh[0:1, 0:1], 1.0)
                nc.vector.tensor_scalar(out=oneh[:], in0=oneh[:], scalar1=invp[:], scalar2=None, op0=AluOpType.mult)
                nc.vector.tensor_tensor(out=psel[:], in0=psel[:], in1=oneh[:], op=AluOpType.add)

                # tsel_half = ith if n_it >= 500 else ones
                flagt = cp.tile([128, 1], f32)
                nc.vector.tensor_scalar(out=flagt[:], in0=c2a[:, 1:2], scalar1=MIN_PTS, scalar2=None, op0=AluOpType.is_ge)
                invt = cp.tile([128, 1], f32)
                nc.vector.tensor_scalar(out=invt[:], in0=flagt[:], scalar1=-1.0, scalar2=None, op0=AluOpType.mult)
                nc.vector.tensor_scalar(out=invt[:], in0=invt[:], scalar1=1.0, scalar2=None, op0=AluOpType.add)
                tsel = wp.tile([128, AJ], f32)
                nc.vector.tensor_scalar(out=tsel[:], in0=ith[:], scalar1=flagt[:], scalar2=None, op0=AluOpType.mult)
                nc.vector.tensor_scalar(out=tsel[:], in0=tsel[:], scalar1=invt[:], scalar2=None, op0=AluOpType.add)

                # combined rhs row: |t|^2 + (1-tsel)*BIG   (pm layout)
                cmb = cp.tile([128, AJ], f32)
                nc.vector.tensor_scalar(out=cmb[:], in0=tsel[:], scalar1=-float(BIG), scalar2=None, op0=AluOpType.mult)
                nc.vector.tensor_scalar(out=cmb[:], in0=cmb[:], scalar1=float(BIG), scalar2=None, op0=AluOpType.add)
                nc.vector.tensor_tensor(out=cmb[:], in0=cmb[:], in1=ttpm[:], op=AluOpType.add)
                # 3-term bf16 split of |t|^2+mask -> staging rows 18-20
                w1, w2, w3 = split3(cmb[:], AJ, "w")
                nc.sync.dma_start(stage_r[18:19, :], w1[:])
                nc.sync.dma_start(stage_r[19:20, :], w2[:])
                nc.sync.dma_start(stage_r[20:21, :], w3[:])
                rhs_f = cp.tile([21, NJ], f32)
                nc.sync.dma_start(rhs_f[:], stage_r[:])
                rhs_bf = cp.tile([21, NJ], bf16)
                nc.vector.tensor_copy(rhs_bf[:], rhs_f[:])

                # n_sel and threshold index k = 1 + (n_sel >> 1)
                nsp = wp.tile([128, 1], f32)
                nc.vector.tensor_reduce(nsp[:], psel[:], axis=AX.X, op=AluOpType.add)
                ns_ps = psp.tile([128, 1], f32)
                nc.tensor.matmul(ns_ps[:], lhsT=ones[:], rhs=nsp[:], start=True, stop=True)
                nsa = cp.tile([128, 1], f32)
                nc.vector.tensor_copy(nsa[:], ns_ps[:])
                ns_i = wp.tile([128, 1], i32)
                nc.vector.tensor_copy(ns_i[:], nsa[:])
                kk_i = cp.tile([128, 1], i32)
                nc.vector.tensor_scalar(out=kk_i[:], in0=ns_i[:], scalar1=1, scalar2=None, op0=AluOpType.logical_shift_right)
                nc.vector.tensor_scalar(out=kk_i[:], in0=kk_i[:], scalar1=1, scalar2=None, op0=AluOpType.add)
                kk_f = cp.tile([128, 1], f32)
                nc.vector.tensor_copy(kk_f[:], kk_i[:])

            # ---------- main loop: 55 i-tiles x 8 matmuls(N=448), reduce-min ----------
            pm2 = cp.tile([128, AI, 2], f32)
            with tc.tile_pool(name='ps_main', bufs=2, space='PSUM') as psm:
                for it in range(AI):
                    lT = lhsT_bf[:, it * 128:(it + 1) * 128]
                    for g in range(2):
                        pst = psm.tile([128, 4, 512], f32, tag="mm")
                        for s in range(4):
                            j0 = (g * 4 + s) * JT
                            nc.tensor.matmul(pst[:, s, 0:JT], lhsT=lT, rhs=rhs_bf[:, j0:j0 + JT], start=True, stop=True)
                        nc.vector.tensor_reduce(pm2[:, it, g:g + 1],
                                                pst[:, :, 0:JT], axis=AX.XY, op=AluOpType.min)

            # ---------- diff = relu(min + |p|^2); pair AllReduce(min) ----------
            pm = wp.tile([128, AI], f32)
            nc.vector.tensor_reduce(pm[:], pm2[:], axis=AX.X, op=AluOpType.min)
            diff0 = wp.tile([128, AI], f32)
            nc.vector.tensor_tensor(out=diff0[:], in0=pm[:], in1=pp[:], op=AluOpType.add)
            nc.vector.tensor_scalar(out=diff0[:], in0=diff0[:], scalar1=0.0, scalar2=None, op0=AluOpType.max)

            cc1i = dp.tile([128, AI], f32)
            cc1o = dp.tile([128, AI], f32)
            nc.sync.dma_start(cc1i[:], diff0[:])
            nc.gpsimd.collective_compute(
                "AllReduce", AluOpType.min,
                replica_groups=[[0, 1], [2, 3], [4, 5], [6, 7]],
                ins=[cc1i[:]], outs=[cc1o[:]])
            diff = cp.tile([128, AI], f32)
            nc.sync.dma_start(diff[:], cc1o[:])
            nc.sync.dma_start(dbg_diff[:], diff[:])

            # ---------- diff_s bits, split high-23 / low-8 ----------
            ds = wp.tile([128, AI], f32)
            nc.vector.tensor_tensor(out=ds[:], in0=diff[:], in1=psel[:], op=AluOpType.mult)
            bigp = wp.tile([128, AI], f32)
            nc.vector.tensor_scalar(out=bigp[:], in0=psel[:], scalar1=-float(BIG), scalar2=None, op0=AluOpType.mult)
            nc.vector.tensor_scalar(out=bigp[:], in0=bigp[:], scalar1=float(BIG), scalar2=None, op0=AluOpType.add)
            nc.vector.tensor_tensor(out=ds[:], in0=ds[:], in1=bigp[:], op=AluOpType.add)
            ds_i = wp.tile([128, AI], i32)
            nc.vector.tensor_copy(ds_i[:], ds[:].bitcast(i32))
            hb = cp.tile([128, AI], i32)
            nc.vector.tensor_scalar(out=hb[:], in0=ds_i[:], scalar1=8, scalar2=None, op0=AluOpType.logical_shift_right)
            lb = cp.tile([128, AI], i32)
            nc.vector.tensor_scalar(out=lb[:], in0=ds_i[:], scalar1=255, scalar2=None, op0=AluOpType.bitwise_and)

            # ---------- exact k-select via 16-ary bisection ----------
            iot = cp.tile([128, 15], i32)
            nc.gpsimd.iota(iot[:], pattern=[[1, 15]], base=1, channel_multiplier=0)

            with tc.tile_pool(name='ps_sel', bufs=2, space='PSUM') as pss, \
                 tc.tile_pool(name='selw', bufs=2) as sw:

                def kselect(data_i, kf, hi_init, nrounds, tagn):
                    lo = sw.tile([128, 1], i32, name=f"lo_{tagn}")
                    hi = sw.tile([128, 1], i32, name=f"hi_{tagn}")
                    nc.vector.memset(lo[:], 0)
                    nc.vector.memset(hi[:], hi_init)
                    for r in range(nrounds):
                        st = sw.tile([128, 1], i32, name=f"st_{tagn}", tag=f"st{tagn}")
                        nc.vector.tensor_tensor(out=st[:], in0=hi[:], in1=lo[:], op=AluOpType.subtract)
                        nc.vector.tensor_scalar(out=st[:], in0=st[:], scalar1=4, scalar2=None, op0=AluOpType.logical_shift_right)
                        nc.vector.tensor_scalar(out=st[:], in0=st[:], scalar1=1, scalar2=None, op0=AluOpType.max)
                        pr = sw.tile([128, 15], i32, name=f"pr_{tagn}", tag=f"pr{tagn}")
                        nc.vector.tensor_tensor(out=pr[:], in0=iot[:], in1=st[:].broadcast_to([128, 15]), op=AluOpType.mult)
                        nc.vector.tensor_tensor(out=pr[:], in0=pr[:], in1=lo[:].broadcast_to([128, 15]), op=AluOpType.add)
                        cmp = sw.tile([128, 15, AI], f32, name=f"cmp_{tagn}", tag=f"cmp{tagn}")
                        nc.vector.tensor_tensor(out=cmp[:],
                                                in0=data_i[:, None, :].broadcast_to([128, 15, AI]),
                                                in1=pr[:, :, None].broadcast_to([128, 15, AI]),
                                                op=AluOpType.is_lt)
                        pcnt = sw.tile([128, 15], f32, name=f"pc_{tagn}", tag=f"pc{tagn}")
                        nc.vector.tensor_reduce(pcnt[:], cmp[:], axis=AX.X, op=AluOpType.add)
                        ct_ps = pss.tile([128, 15], f32, name=f"ct_{tagn}", tag=f"ct{tagn}")
                        nc.tensor.matmul(ct_ps[:], lhsT=ones[:], rhs=pcnt[:], start=True, stop=True)
                        cnt = sw.tile([128, 15], f32, name=f"cnt_{tagn}", tag=f"cnt{tagn}")
                        nc.vector.tensor_copy(cnt[:], ct_ps[:])
                        flag = sw.tile([128, 15], f32, name=f"fl_{tagn}", tag=f"fl{tagn}")
                        nc.vector.tensor_tensor(out=flag[:], in0=cnt[:], in1=kf[:].broadcast_to([128, 15]), op=AluOpType.is_ge)
                        flag_i = sw.tile([128, 15], i32, name=f"fli_{tagn}", tag=f"fli{tagn}")
                        nc.vector.tensor_copy(flag_i[:], flag[:])
                        inv_i = sw.tile([128, 15], i32, name=f"inv_{tagn}", tag=f"inv{tagn}")
                        nc.vector.tensor_scalar(out=inv_i[:], in0=flag_i[:], scalar1=-1, scalar2=None, op0=AluOpType.mult)
                        nc.vector.tensor_scalar(out=inv_i[:], in0=inv_i[:], scalar1=1, scalar2=None, op0=AluOpType.add)
                        # new lo = max(lo, max(probe*inv))
                        nlc = sw.tile([128, 15], i32, name=f"nlc_{tagn}", tag=f"nlc{tagn}")
                        nc.vector.tensor_tensor(out=nlc[:], in0=pr[:], in1=inv_i[:], op=AluOpType.mult)
                        nl = sw.tile([128, 1], i32, name=f"nl_{tagn}", tag=f"nl{tagn}")
                        nc.vector.tensor_reduce(nl[:], nlc[:], axis=AX.X, op=AluOpType.max)
                        nc.vector.tensor_tensor(out=lo[:], in0=lo[:], in1=nl[:], op=AluOpType.max)
                        # new hi = min(hi, min(probe*flag + inv*HUGE))
                        nc.vector.tensor_tensor(out=nlc[:], in0=pr[:], in1=flag_i[:], op=AluOpType.mult)
                        nc.vector.tensor_scalar(out=inv_i[:], in0=inv_i[:], scalar1=HB_HI + 16, scalar2=None, op0=AluOpType.mult)
                        nc.vector.tensor_tensor(out=nlc[:], in0=nlc[:], in1=inv_i[:], op=AluOpType.add)
                        nh = sw.tile([128, 1], i32, name=f"nh_{tagn}", tag=f"nh{tagn}")
                        nc.vector.tensor_reduce(nh[:], nlc[:], axis=AX.X, op=AluOpType.min)
                        nc.vector.tensor_tensor(out=hi[:], in0=hi[:], in1=nh[:], op=AluOpType.min)
                    return lo

                bstar = kselect(hb, kk_f, HB_HI, 7, "h")          # high-23 bits of thr

                # r1 = count(hb < B*), k2 = k - r1
                cmpb = sw.tile([128, AI], f32)
                nc.vector.tensor_tensor(out=cmpb[:], in0=hb[:], in1=bstar[:].broadcast_to([128, AI]), op=AluOpType.is_lt)
                r1p = sw.tile([128, 1], f32)
                nc.vector.tensor_reduce(r1p[:], cmpb[:], axis=AX.X, op=AluOpType.add)
                r1_ps = pss.tile([128, 1], f32)
                nc.tensor.matmul(r1_ps[:], lhsT=ones[:], rhs=r1p[:], start=True, stop=True)
                r1 = sw.tile([128, 1], f32)
                nc.vector.tensor_copy(r1[:], r1_ps[:])
                k2f = sw.tile([128, 1], f32)
                nc.vector.tensor_tensor(out=k2f[:], in0=kk_f[:], in1=r1[:], op=AluOpType.subtract)

                # cand = lb where hb==B* else 256
                eqb = sw.tile([128, AI], i32)
                nc.vector.tensor_tensor(out=eqb[:], in0=hb[:], in1=bstar[:].broadcast_to([128, AI]), op=AluOpType.is_equal)
                cand = sw.tile([128, AI], i32)
                nc.vector.tensor_tensor(out=cand[:], in0=lb[:], in1=eqb[:], op=AluOpType.mult)
                inv2 = sw.tile([128, AI], i32)
                nc.vector.tensor_scalar(out=inv2[:], in0=eqb[:], scalar1=-1, scalar2=None, op0=AluOpType.mult)
                nc.vector.tensor_scalar(out=inv2[:], in0=inv2[:], scalar1=1, scalar2=None, op0=AluOpType.add)
                nc.vector.tensor_scalar(out=inv2[:], in0=inv2[:], scalar1=256, scalar2=None, op0=AluOpType.mult)
                nc.vector.tensor_tensor(out=cand[:], in0=cand[:], in1=inv2[:], op=AluOpType.add)

                lstar = kselect(cand, k2f, 257, 4, "l")           # low-8 bits of thr

                # keep = (hb < B*) | (cand < L*)   (disjoint)
                keep = sw.tile([128, AI], f32)
                cl = sw.tile([128, AI], f32)
                nc.vector.tensor_tensor(out=cl[:], in0=cand[:], in1=lstar[:].broadcast_to([128, AI]), op=AluOpType.is_lt)
                nc.vector.tensor_tensor(out=keep[:], in0=cmpb[:], in1=cl[:], op=AluOpType.add)

                # ---------- final loss ----------
                mk = sw.tile([128, AI], f32)
                nc.vector.tensor_tensor(out=mk[:], in0=keep[:], in1=mnat[:], op=AluOpType.mult)
                d2 = sw.tile([128, AI], f32)
                nc.vector.tensor_tensor(out=d2[:], in0=diff[:], in1=diff[:], op=AluOpType.mult)
                nc.vector.tensor_tensor(out=d2[:], in0=d2[:], in1=mk[:], op=AluOpType.mult)
                s2 = sw.tile([128, 2], f32)
                nc.vector.tensor_reduce(s2[:, 0:1], d2[:], axis=AX.X, op=AluOpType.add)
                nc.vector.tensor_reduce(s2[:, 1:2], mk[:], axis=AX.X, op=AluOpType.add)
                s2_ps = pss.tile([128, 2], f32)
                nc.tensor.matmul(s2_ps[:], lhsT=ones[:], rhs=s2[:], start=True, stop=True)
                s2a = sw.tile([128, 2], f32)
                nc.vector.tensor_copy(s2a[:], s2_ps[:])
                den = sw.tile([128, 1], f32)
                nc.vector.tensor_scalar(out=den[:], in0=s2a[:, 1:2], scalar1=1e-12, scalar2=None, op0=AluOpType.add)
                rden = sw.tile([128, 1], f32)
                nc.vector.reciprocal(rden[:], den[:])
                lb_t = sw.tile([128, 1], f32)
                nc.vector.tensor_tensor(out=lb_t[:], in0=s2a[:, 0:1], in1=rden[:], op=AluOpType.mult)
                nc.vector.tensor_scalar(out=lb_t[:], in0=lb_t[:], scalar1=0.125, scalar2=None, op0=AluOpType.mult)

                # global mean over batches: AllReduce(add) of loss_b/8 over 8 cores
                cc2i = dp.tile([1, 1], f32)
                cc2o = dp.tile([1, 1], f32)
                nc.sync.dma_start(cc2i[:], lb_t[0:1, 0:1])
                nc.gpsimd.collective_compute(
                    "AllReduce", AluOpType.add,
                    replica_groups=[[0, 1, 2, 3, 4, 5, 6, 7]],
                    ins=[cc2i[:]], outs=[cc2o[:]])
                lossg = sw.tile([1, 1], f32)
                nc.sync.dma_start(lossg[:], cc2o[:])

                # out = exp(-alpha) * loss + alpha
                ea = sw.tile([1, 1], f32)
                nc.scalar.activation(ea[:], alph[:], AF.Exp, scale=-1.0)
                ov = sw.tile([1, 1], f32)
                nc.vector.tensor_tensor(out=ov[:], in0=ea[:], in1=lossg[:], op=AluOpType.mult)
                nc.vector.tensor_tensor(out=ov[:], in0=ov[:], in1=alph[:], op=AluOpType.add)
                nc.sync.dma_start(out_d[:], ov[:])

                # debug row: n_ip, n_it, n_sel, k, B*, L*, r1, loss_b*8... (per-partition col dump)
                dbgt = sw.tile([128, 8], f32)
                nc.vector.tensor_copy(dbgt[:, 0:1], c2a[:, 0:1])
                nc.vector.tensor_copy(dbgt[:, 1:2], c2a[:, 1:2])
                nc.vector.tensor_copy(dbgt[:, 2:3], nsa[:])
                nc.vector.tensor_copy(dbgt[:, 3:4], kk_f[:])
                nc.vector.tensor_copy(dbgt[:, 4:5], bstar[:])
                nc.vector.tensor_copy(dbgt[:, 5:6], lstar[:])
                nc.vector.tensor_copy(dbgt[:, 6:7], r1[:])
                nc.vector.tensor_copy(dbgt[:, 7:8], lb_t[:])
                nc.sync.dma_start(dbg_d[:], dbgt[:])

    return nc


# --------------------------------------------------------------------------
# host wrapper
# --------------------------------------------------------------------------
_NC_CACHE = {}


def _get_nc():
    if 'nc' not in _NC_CACHE:
        _NC_CACHE['nc'] = build_nc()
    return _NC_CACHE['nc']


def _marshal(prediction_tensor, target_tensor, mask, alpha):
    pred = np.asarray(prediction_tensor, np.float32)
    tgt = np.asarray(target_tensor, np.float32)
    msk = np.asarray(mask, np.float32)
    al = np.asarray(alpha, np.float32).reshape(1, 1)

    in_maps = []
    for c in range(N_CORES):
        b, h = c // 2, c % 2
        p = np.empty((NI, 3), np.float32)
        p[:N] = pred[b]
        p[N:] = pred[b, 0]
        t = np.full((NI, 3), PADV, np.float32)
        t[:N] = tgt[b]
        th = np.full((NJ, 3), PADV, np.float32)
        th[:MH] = tgt[b, h * MH:(h + 1) * MH]
        m = np.zeros(NI, np.float32)
        m[:N] = msk[b]
        in_maps.append({
            'pred_pm': np.ascontiguousarray(p.reshape(128, AI * 3)),
            'pred_nat': np.ascontiguousarray(
                p.reshape(AI, 128, 3).transpose(1, 0, 2).reshape(128, AI * 3)),
            'tgt_nat': np.ascontiguousarray(
                t.reshape(AI, 128, 3).transpose(1, 0, 2).reshape(128, AI * 3)),
            'tgt_half_pm': np.ascontiguousarray(th.reshape(128, AJ * 3)),
            'mask_nat': np.ascontiguousarray(m.reshape(AI, 128).T),
            'valid_nat': np.ascontiguousarray(
                (np.arange(NI) < N).astype(np.float32).reshape(AI, 128).T),
            'alpha_in': al,
        })
    return in_maps


def run_cores(prediction_tensor, target_tensor, mask, alpha, **rb_kwargs):
    nc = _get_nc()
    in_maps = _marshal(prediction_tensor, target_tensor, mask, alpha)
    return run_bass_kernel_spmd(nc, in_maps, core_ids=list(range(N_CORES)), **rb_kwargs)


def kernel(prediction_tensor, target_tensor, mask, alpha):
    res = run_cores(prediction_tensor, target_tensor, mask, alpha)
    return res.results[0]['out'].reshape(1).astype(np.float32)


# revision 20
# speedup vs baseline: 1.1110x; 1.1110x over previous
"""Chamfer L2 loss (nn_ChamferL2Loss) Trainium2 Bass kernel.

Strategy: 8 NeuronCores, core c handles batch b=c//2 and target-half h=c%2.
Each core computes row-mins of the [7000 x 3500] squared-distance matrix for
its half via K=4 matmuls (coords + fused |t|^2 + column-mask row), DVE
reduce-min from PSUM, then an AllReduce(min) within core pairs merges halves.
The kth-value threshold (jnp.sort + take in the reference) is found with an
exact bit-pattern multi-way bisection (high-23/low-8 bit split keeps all DVE
integer arithmetic within fp32-exact range).  Final per-batch losses are
averaged with an 8-core AllReduce(add).
"""

import numpy as np

import concourse.bass as bass
import concourse.tile as tile
import concourse.mybir as mybir
from concourse.alu_op_type import AluOpType
from concourse.vector_clock import ScopedClock
from concourse.bass_utils import run_bass_kernel_spmd

f32 = mybir.dt.float32
bf16 = mybir.dt.bfloat16
i32 = mybir.dt.int32
AX = mybir.AxisListType
AF = mybir.ActivationFunctionType

B = 4
N = 7000          # points per cloud
NI = 7040         # padded rows (55 * 128)
AI = 55           # NI / 128
MH = 3500         # targets per core (half)
NJ = 3584         # padded cols (28 * 128 = 8 * 448)
AJ = 28           # NJ / 128
JT = 448          # matmul free-dim tile
BIG = np.float32(1e10)
PADV = np.float32(1e4)
MARGIN = 0.05
MIN_PTS = 500.0
HB_HI = 8388609   # 2^23 + 1: exclusive upper bound for high-23-bit patterns

N_CORES = 8


# --------------------------------------------------------------------------
# Custom DVE op: out = min(in0, in1); accum_out = min(C0, min_k out[k]).
# Consumes two tiles per instruction (both DVE read ports), halving the
# per-element cost of the row-min versus tensor_reduce.  Registered via the
# documented extension point in concourse.dve_ops (define + append to OPS).
# --------------------------------------------------------------------------
def _register_minmin():
    from concourse import dve_ops
    from concourse.dve_spec import Spec, Src0, Src1, C0, minn
    name = "TT_MIN_REDUCE_ANT"
    for o in dve_ops.OPS:
        if o.name == name:
            return o
    op = dve_ops.DveOp(
        name,
        Spec(body=minn(Src0, Src1), accum=minn, accum_init=C0,
             reference=lambda in0, in1, c0, c1, c2: np.minimum(
                 in0.astype(np.float32), in1.astype(np.float32))),
        subdim=False,
        uops_sha={"v3": "80668f319ac378ba", "v4": "23f6c1536de15f6a"},
    )
    dve_ops.OPS.append(op)
    dve_ops.CUSTOM_DVE_SPECS[name] = op.spec
    dve_ops._SUB_OPCODE_FOR_NAME[name] = max(dve_ops._SUB_OPCODE_FOR_NAME.values()) + 1
    assert dve_ops._SUB_OPCODE_FOR_NAME[name] < 0x20
    return op


MINMIN = _register_minmin()


# --------------------------------------------------------------------------
# TileContext workaround: this container's walrus build rejects instructions
# carrying more than one semaphore wait ("Too many sync wait commands").
# Split extra waits onto single-wait NOPs inserted just before the holder.
# --------------------------------------------------------------------------
def _split_multiwaits(nc, max_waits=1):
    for f in nc.m.functions:
        for bb in f.blocks:
            insts = bb.instructions
            idx = 0
            while idx < len(insts):
                inst = insts[idx]
                si = inst.sync_info
                if si is not None and len(si.on_wait) > max_waits:
                    waits = list(si.on_wait)
                    inst.sync_info = mybir.SyncInfo(
                        on_wait=waits[:max_waits], on_update=list(si.on_update))
                    for w in waits[max_waits:]:
                        nop = mybir.InstNoOp(
                            name=f"waitsplit-{nc.next_id()}", ins=[], outs=[])
                        nop.engine = inst.engine
                        nop.sync_info = mybir.SyncInfo(on_wait=[w], on_update=[])
                        nc.register_instruction(nop)
                        insts.insert(idx, nop)
                        idx += 1
                idx += 1


class TC(tile.TileContext):
    def schedule_and_allocate(self, validate_deps=False):
        r = super().schedule_and_allocate(validate_deps=validate_deps)
        _split_multiwaits(self.nc)
        return r


# --------------------------------------------------------------------------
# device program
# --------------------------------------------------------------------------
def _ptree_fold32(nc, pool, src, op):
    """Reduce [128, F] across partitions to [32, F] via 2 pairwise folds
    (engine SBUF accesses must start at 32-aligned partitions)."""
    f = src.shape[-1]
    h64 = pool.tile([64, f], f32, name=f"foldc64_{nc.next_id()}")
    nc.vector.tensor_copy(h64[:], src[64:128, :])
    t64 = pool.tile([64, f], f32, name=f"fold64_{nc.next_id()}")
    nc.vector.tensor_tensor(out=t64[:], in0=src[0:64, :], in1=h64[:], op=op)
    h32 = pool.tile([32, f], f32, name=f"foldc32_{nc.next_id()}")
    nc.vector.tensor_copy(h32[:], t64[32:64, :])
    t32 = pool.tile([32, f], f32, name=f"fold32_{nc.next_id()}")
    nc.vector.tensor_tensor(out=t32[:], in0=t64[0:32, :], in1=h32[:], op=op)
    return t32


def build_nc():
    nc = bass.Bass(num_devices=N_CORES)

    pred_pm = nc.declare_dram_parameter('pred_pm', [128, AI * 3], f32, isOutput=False)
    pred_nat = nc.declare_dram_parameter('pred_nat', [128, AI * 3], f32, isOutput=False)
    tgt_nat = nc.declare_dram_parameter('tgt_nat', [128, AI * 3], f32, isOutput=False)
    tgt_half_pm = nc.declare_dram_parameter('tgt_half_pm', [128, AJ * 3], f32, isOutput=False)
    mask_nat = nc.declare_dram_parameter('mask_nat', [128, AI], f32, isOutput=False)
    valid_nat = nc.declare_dram_parameter('valid_nat', [128, AI], f32, isOutput=False)
    alpha_in = nc.declare_dram_parameter('alpha_in', [1, 1], f32, isOutput=False)

    out_d = nc.declare_dram_parameter('out', [1, 1], f32, isOutput=True)
    dbg_d = nc.declare_dram_parameter('dbg', [128, 8], f32, isOutput=True)
    dbg2_d = nc.declare_dram_parameter('dbg2', [128, 8], f32, isOutput=True)
    dbg_diff = nc.declare_dram_parameter('dbg_diff', [128, AI], f32, isOutput=True)

    with TC(nc) as tc:
        with tc.tile_pool(name='const', bufs=1) as cp, \
             tc.tile_pool(name='work', bufs=2) as wp, \
             tc.tile_pool(name='dram', bufs=1, space='DRAM') as dp:

            # ---------- loads ----------
            # bf16 split-precision matmul, K=21:
            #   lhsT rows: P1 P1 P1 P2 P2 P3 (x3 coords) + three ones rows
            #   rhs rows:  V1 V2 V3 V1 V2 V1 (x3 coords, V=-2t) + w1 w2 w3
            # where X = X1+X2+X3 is a 3-term bf16 split and w is the 3-term
            # split of |t|^2 + (1-tsel)*BIG.  Dropped cross terms are
            # O(|p||t| 2^-26).  Rows are assembled via a DRAM staging
            # buffer (engine writes must start at 32-aligned partitions,
            # DMA round-trip through DRAM sidesteps that).
            ppm = cp.tile([128, AI * 3], f32)
            nc.gpsimd.dma_start(ppm[:], pred_pm[:])

            pnat = cp.tile([128, AI * 3], f32)
            nc.gpsimd.dma_start(pnat[:], pred_nat[:])
            tnat = cp.tile([128, AI * 3], f32)
            nc.gpsimd.dma_start(tnat[:], tgt_nat[:])
            thpm = cp.tile([128, AJ * 3], f32)
            nc.gpsimd.dma_start(thpm[:], tgt_half_pm[:])
            mnat = cp.tile([128, AI], f32)
            nc.gpsimd.dma_start(mnat[:], mask_nat[:])
            vnat = cp.tile([128, AI], f32)
            nc.gpsimd.dma_start(vnat[:], valid_nat[:])
            alph = cp.tile([1, 1], f32)
            nc.sync.dma_start(alph[:], alpha_in[:])

            ones = cp.tile([128, 128], f32)
            nc.vector.memset(ones[:], 1.0)

            pnat3 = pnat[:].rearrange("p (a k) -> p a k", k=3)
            tnat3 = tnat[:].rearrange("p (a k) -> p a k", k=3)
            thpm3 = thpm[:].rearrange("p (a k) -> p a k", k=3)

            ppm3 = ppm[:].rearrange("p (a k) -> p a k", k=3)

            stage_l = dp.tile([21, NI], f32)
            stage_r = dp.tile([21, NJ], f32)
            onesAI = wp.tile([128, AI], f32)
            nc.vector.memset(onesAI[:], 1.0)

            def cdu(dst, src_ap, cols, tagn):
                # dst <- f32(bf16(src)): round-trip through bf16
                tmpb = wp.tile([128, cols], bf16, name=f"cdub_{nc.next_id()}", tag=f"cdub{tagn}")
                nc.vector.tensor_copy(tmpb[:], src_ap)
                nc.vector.tensor_copy(dst[:], tmpb[:])

            def split3(src_ap, cols, tagn):
                s1 = wp.tile([128, cols], f32, name=f"s1_{nc.next_id()}", tag=f"s1{tagn}")
                s2 = wp.tile([128, cols], f32, name=f"s2_{nc.next_id()}", tag=f"s2{tagn}")
                s3 = wp.tile([128, cols], f32, name=f"s3_{nc.next_id()}", tag=f"s3{tagn}")
                r = wp.tile([128, cols], f32, name=f"r_{nc.next_id()}", tag=f"r{tagn}")
                cdu(s1, src_ap, cols, tagn)
                nc.vector.tensor_tensor(out=r[:], in0=src_ap, in1=s1[:], op=AluOpType.subtract)
                cdu(s2, r[:], cols, tagn)
                nc.vector.tensor_tensor(out=r[:], in0=r[:], in1=s2[:], op=AluOpType.subtract)
                cdu(s3, r[:], cols, tagn)
                return s1, s2, s3

            # lhsT planes: pred splits (pm layout, point = p*AI + a)
            for k in range(3):
                p1, p2, p3 = split3(ppm3[:, :, k], AI, "p")
                for row, t in ((0, p1), (3, p1), (6, p1), (9, p2), (12, p2), (15, p3)):
                    nc.scalar.dma_start(stage_l[row + k:row + k + 1, :], t[:])
            for row in (18, 19, 20):
                nc.scalar.dma_start(stage_l[row:row + 1, :], onesAI[:])

            # rhs coordinate planes: V = -2*t splits (pm layout, point = p*AJ + a)
            for k in range(3):
                vneg = wp.tile([128, AJ], f32, name=f"vneg_{k}", tag="vneg")
                nc.vector.tensor_scalar(out=vneg[:], in0=thpm3[:, :, k], scalar1=-2.0, scalar2=None, op0=AluOpType.mult)
                t1, t2, t3 = split3(vneg[:], AJ, "t")
                for row, t in ((0, t1), (3, t2), (6, t3), (9, t1), (12, t2), (15, t1)):
                    nc.sync.dma_start(stage_r[row + k:row + k + 1, :], t[:])

            lhsT_f = cp.tile([21, NI], f32)
            nc.scalar.dma_start(lhsT_f[:], stage_l[:])
            lhsT_bf = cp.tile([21, NI], bf16)
            nc.vector.tensor_copy(lhsT_bf[:], lhsT_f[:])

            # ---------- |p|^2 per point (nat layout), |t|^2 per half target (pm layout)
            sqp = wp.tile([128, AI * 3], f32)
            nc.vector.tensor_tensor(out=sqp[:], in0=pnat[:], in1=pnat[:], op=AluOpType.mult)
            sqp3 = sqp[:].rearrange("p (a k) -> p a k", k=3)
            pp = cp.tile([128, AI], f32)
            nc.vector.tensor_tensor(out=pp[:], in0=sqp3[:, :, 0], in1=sqp3[:, :, 1], op=AluOpType.add)
            nc.vector.tensor_tensor(out=pp[:], in0=pp[:], in1=sqp3[:, :, 2], op=AluOpType.add)

            sqt = wp.tile([128, AJ * 3], f32)
            nc.vector.tensor_tensor(out=sqt[:], in0=thpm[:], in1=thpm[:], op=AluOpType.mult)
            sqt3 = sqt[:].rearrange("p (a k) -> p a k", k=3)
            ttpm = cp.tile([128, AJ], f32)
            nc.vector.tensor_tensor(out=ttpm[:], in0=sqt3[:, :, 0], in1=sqt3[:, :, 1], op=AluOpType.add)
            nc.vector.tensor_tensor(out=ttpm[:], in0=ttpm[:], in1=sqt3[:, :, 2], op=AluOpType.add)

            # ---------- bounds from pred (exact min/max over the 7000 real rows)
            # pad rows hold PADV=1e4 > any real coord, fine for max; for min
            # they lose to real values (real coords ~N(0,1), all < 1e4).
            # PADV would corrupt min only if all values padded - not the case.
            mx32 = _ptree_fold32(nc, wp, pnat[:], AluOpType.max)   # [32, 165]
            mn32 = _ptree_fold32(nc, wp, pnat[:], AluOpType.min)   # [32, 165]
            mxc = wp.tile([32, 3], f32)
            mnc = wp.tile([32, 3], f32)
            mx32v = mx32[:].rearrange("p (a k) -> p k a", k=3)
            mn32v = mn32[:].rearrange("p (a k) -> p k a", k=3)
            nc.vector.tensor_reduce(mxc[:], mx32v, axis=AX.X, op=AluOpType.max)
            nc.vector.tensor_reduce(mnc[:], mn32v, axis=AX.X, op=AluOpType.min)
            mxf = wp.tile([1, 96], f32)
            mnf = wp.tile([1, 96], f32)
            nc.scalar.dma_start(mxf[:], mxc[:])
            nc.scalar.dma_start(mnf[:], mnc[:])
            mx13 = wp.tile([1, 3], f32)
            mn13 = wp.tile([1, 3], f32)
            nc.vector.tensor_reduce(mx13[:], mxf[:].rearrange("o (g k) -> o k g", k=3), axis=AX.X, op=AluOpType.max)
            nc.vector.tensor_reduce(mn13[:], mnf[:].rearrange("o (g k) -> o k g", k=3), axis=AX.X, op=AluOpType.min)

            # lo = mn + 0.05*w ; hi = mx - 0.05*w ; w = mx - mn     (f32, as ref)
            w13 = wp.tile([1, 3], f32)
            nc.vector.tensor_tensor(out=w13[:], in0=mx13[:], in1=mn13[:], op=AluOpType.subtract)
            mw = wp.tile([1, 3], f32)
            nc.vector.tensor_scalar(out=mw[:], in0=w13[:], scalar1=float(MARGIN), scalar2=None, op0=AluOpType.mult)
            lo13 = wp.tile([1, 3], f32)
            nc.vector.tensor_tensor(out=lo13[:], in0=mn13[:], in1=mw[:], op=AluOpType.add)
            hi13 = wp.tile([1, 3], f32)
            nc.vector.tensor_tensor(out=hi13[:], in0=mx13[:], in1=mw[:], op=AluOpType.subtract)
            hl13 = wp.tile([1, 3], f32)
            nc.vector.tensor_tensor(out=hl13[:], in0=hi13[:], in1=lo13[:], op=AluOpType.subtract)
            # r_lo = (hi-lo)*bi*bs + lo ; r_hi = r_lo + (hi-lo)*bs
            bibs = wp.tile([1, 3], f32)   # bi*bs = [0.4, 0, 0]
            nc.vector.memset(bibs[:], 0.0)
            nc.vector.memset(bibs[0:1, 0:1], 0.4)
            bs13 = wp.tile([1, 3], f32)   # bs = [0.1, 1, 1]
            nc.vector.memset(bs13[:], 1.0)
            nc.vector.memset(bs13[0:1, 0:1], 0.1)
            t13 = wp.tile([1, 3], f32)
            nc.vector.tensor_tensor(out=t13[:], in0=hl13[:], in1=bibs[:], op=AluOpType.mult)
            rlo13 = wp.tile([1, 6], f32)
            nc.vector.tensor_tensor(out=rlo13[:, 0:3], in0=t13[:], in1=lo13[:], op=AluOpType.add)
            nc.vector.tensor_tensor(out=t13[:], in0=hl13[:], in1=bs13[:], op=AluOpType.mult)
            nc.vector.tensor_tensor(out=rlo13[:, 3:6], in0=rlo13[:, 0:3], in1=t13[:], op=AluOpType.add)

            # broadcast [1,6] -> [128,6] via K=1 matmul with ones
            with tc.tile_pool(name='ps_pre', bufs=1, space='PSUM') as psp:
                rl_ps = psp.tile([128, 6], f32)
                nc.tensor.matmul(rl_ps[:], lhsT=ones[0:1, :], rhs=rlo13[:], start=True, stop=True)
                rlh = cp.tile([128, 6], f32)
                nc.vector.tensor_copy(rlh[:], rl_ps[:])

                # ---------- indicators (strict > r_lo and < r_hi on all 3 dims)
                def indicator(dst, src3, acols):
                    tmp = wp.tile([128, acols], f32, name=f"indt_{nc.next_id()}", tag="indt")
                    for k in range(3):
                        nc.vector.tensor_scalar(out=(dst if k == 0 else tmp)[:, 0:acols], in0=src3[:, :, k],
                                                scalar1=rlh[:, k:k + 1], scalar2=None, op0=AluOpType.is_gt)
                        if k > 0:
                            nc.vector.tensor_tensor(out=dst[:, 0:acols], in0=dst[:, 0:acols], in1=tmp[:, 0:acols], op=AluOpType.mult)
                        nc.vector.tensor_scalar(out=tmp[:, 0:acols], in0=src3[:, :, k],
                                                scalar1=rlh[:, 3 + k:4 + k], scalar2=None, op0=AluOpType.is_lt)
                        nc.vector.tensor_tensor(out=dst[:, 0:acols], in0=dst[:, 0:acols], in1=tmp[:, 0:acols], op=AluOpType.mult)

                ip = cp.tile([128, AI], f32)
                indicator(ip, pnat3, AI)
                # pred_nat pads replicate point 0 (keeps bounds exact), so
                # mask pad rows out of the indicator explicitly
                nc.vector.tensor_tensor(out=ip[:], in0=ip[:], in1=vnat[:], op=AluOpType.mult)
                itf = wp.tile([128, AI], f32)
                indicator(itf, tnat3, AI)
                ith = cp.tile([128, AJ], f32)
                indicator(ith, thpm3, AJ)

                dbg2t = wp.tile([128, 8], f32)
                nc.vector.tensor_copy(dbg2t[:, 0:6], rlh[:])
                nc.vector.tensor_reduce(dbg2t[:, 6:7], ip[:], axis=AX.X, op=AluOpType.add)
                nc.vector.tensor_reduce(dbg2t[:, 7:8], pnat3[:, :, 0], axis=AX.X, op=AluOpType.max)
                nc.sync.dma_start(dbg2_d[:], dbg2t[:])

                # counts over full clouds (pads indicate 0)
                c2 = wp.tile([128, 2], f32)
                nc.vector.tensor_reduce(c2[:, 0:1], ip[:], axis=AX.X, op=AluOpType.add)
                nc.vector.tensor_reduce(c2[:, 1:2], itf[:], axis=AX.X, op=AluOpType.add)
                c2_ps = psp.tile([128, 2], f32)
                nc.tensor.matmul(c2_ps[:], lhsT=ones[:], rhs=c2[:], start=True, stop=True)
                c2a = cp.tile([128, 2], f32)
                nc.vector.tensor_copy(c2a[:], c2_ps[:])

                # psel = ip if n_ip >= 500 else onehot0
                flagp = cp.tile([128, 1], f32)
                nc.vector.tensor_scalar(out=flagp[:], in0=c2a[:, 0:1], scalar1=MIN_PTS, scalar2=None, op0=AluOpType.is_ge)
                invp = cp.tile([128, 1], f32)
                nc.vector.tensor_scalar(out=invp[:], in0=flagp[:], scalar1=-1.0, scalar2=None, op0=AluOpType.mult)
                nc.vector.tensor_scalar(out=invp[:], in0=invp[:], scalar1=1.0, scalar2=None, op0=AluOpType.add)
                psel = cp.tile([128, AI], f32)
                nc.vector.tensor_scalar(out=psel[:], in0=ip[:], scalar1=flagp[:], scalar2=None, op0=AluOpType.mult)
                oneh = wp.tile([128, AI], f32)
                nc.vector.memset(oneh[:], 0.0)
                nc.vector.memset(oneh[0:1, 0:1], 1.0)
                nc.vector.tensor_scalar(out=oneh[:], in0=oneh[:], scalar1=invp[:], scalar2=None, op0=AluOpType.mult)
                nc.vector.tensor_tensor(out=psel[:], in0=psel[:], in1=oneh[:], op=AluOpType.add)

                # tsel_half = ith if n_it >= 500 else ones
                flagt = cp.tile([128, 1], f32)
                nc.vector.tensor_scalar(out=flagt[:], in0=c2a[:, 1:2], scalar1=MIN_PTS, scalar2=None, op0=AluOpType.is_ge)
                invt = cp.tile([128, 1], f32)
                nc.vector.tensor_scalar(out=invt[:], in0=flagt[:], scalar1=-1.0, scalar2=None, op0=AluOpType.mult)
                nc.vector.tensor_scalar(out=invt[:], in0=invt[:], scalar1=1.0, scalar2=None, op0=AluOpType.add)
                tsel = wp.tile([128, AJ], f32)
                nc.vector.tensor_scalar(out=tsel[:], in0=ith[:], scalar1=flagt[:], scalar2=None, op0=AluOpType.mult)
                nc.vector.tensor_scalar(out=tsel[:], in0=tsel[:], scalar1=invt[:], scalar2=None, op0=AluOpType.add)

                # combined rhs row: |t|^2 + (1-tsel)*BIG   (pm layout)
                cmb = cp.tile([128, AJ], f32)
                nc.vector.tensor_scalar(out=cmb[:], in0=tsel[:], scalar1=-float(BIG), scalar2=None, op0=AluOpType.mult)
                nc.vector.tensor_scalar(out=cmb[:], in0=cmb[:], scalar1=float(BIG), scalar2=None, op0=AluOpType.add)
                nc.vector.tensor_tensor(out=cmb[:], in0=cmb[:], in1=ttpm[:], op=AluOpType.add)
                # 3-term bf16 split of |t|^2+mask -> staging rows 18-20
                w1, w2, w3 = split3(cmb[:], AJ, "w")
                nc.sync.dma_start(stage_r[18:19, :], w1[:])
                nc.sync.dma_start(stage_r[19:20, :], w2[:])
                nc.sync.dma_start(stage_r[20:21, :], w3[:])
                rhs_f = cp.tile([21, NJ], f32)
                nc.sync.dma_start(rhs_f[:], stage_r[:])
                rhs_bf = cp.tile([21, NJ], bf16)
                nc.vector.tensor_copy(rhs_bf[:], rhs_f[:])

                # n_sel and threshold index k = 1 + (n_sel >> 1)
                nsp = wp.tile([128, 1], f32)
                nc.vector.tensor_reduce(nsp[:], psel[:], axis=AX.X, op=AluOpType.add)
                ns_ps = psp.tile([128, 1], f32)
                nc.tensor.matmul(ns_ps[:], lhsT=ones[:], rhs=nsp[:], start=True, stop=True)
                nsa = cp.tile([128, 1], f32)
                nc.vector.tensor_copy(nsa[:], ns_ps[:])
                ns_i = wp.tile([128, 1], i32)
                nc.vector.tensor_copy(ns_i[:], nsa[:])
                kk_i = cp.tile([128, 1], i32)
                nc.vector.tensor_scalar(out=kk_i[:], in0=ns_i[:], scalar1=1, scalar2=None, op0=AluOpType.logical_shift_right)
                nc.vector.tensor_scalar(out=kk_i[:], in0=kk_i[:], scalar1=1, scalar2=None, op0=AluOpType.add)
                kk_f = cp.tile([128, 1], f32)
                nc.vector.tensor_copy(kk_f[:], kk_i[:])

            # ---------- main loop: 55 i-tiles x 8 matmuls(N=448), reduce-min ----------
            pm2 = cp.tile([128, AI, 2], f32)
            with tc.tile_pool(name='ps_main', bufs=2, space='PSUM') as psm:
                for it in range(AI):
                    lT = lhsT_bf[:, it * 128:(it + 1) * 128]
                    for g in range(2):
                        pst = psm.tile([128, 4, 512], f32, tag="mm")
                        for s in range(4):
                            j0 = (g * 4 + s) * JT
                            nc.tensor.matmul(pst[:, s, 0:JT], lhsT=lT, rhs=rhs_bf[:, j0:j0 + JT], start=True, stop=True)
                        nc.vector.tensor_reduce(pm2[:, it, g:g + 1],
                                                pst[:, :, 0:JT], axis=AX.XY, op=AluOpType.min)

            # ---------- diff = relu(min + |p|^2); pair AllReduce(min) ----------
            pm = wp.tile([128, AI], f32)
            nc.vector.tensor_reduce(pm[:], pm2[:], axis=AX.X, op=AluOpType.min)
            diff0 = wp.tile([128, AI], f32)
            nc.vector.tensor_tensor(out=diff0[:], in0=pm[:], in1=pp[:], op=AluOpType.add)
            nc.vector.tensor_scalar(out=diff0[:], in0=diff0[:], scalar1=0.0, scalar2=None, op0=AluOpType.max)

            cc1i = dp.tile([128, AI], f32)
            cc1o = dp.tile([128, AI], f32)
            nc.sync.dma_start(cc1i[:], diff0[:])
            nc.gpsimd.collective_compute(
                "AllReduce", AluOpType.min,
                replica_groups=[[0, 1], [2, 3], [4, 5], [6, 7]],
                ins=[cc1i[:]], outs=[cc1o[:]])
            diff = cp.tile([128, AI], f32)
            nc.sync.dma_start(diff[:], cc1o[:])
            nc.sync.dma_start(dbg_diff[:], diff[:])

            # ---------- diff_s bits, split high-23 / low-8 ----------
            ds = wp.tile([128, AI], f32)
            nc.vector.tensor_tensor(out=ds[:], in0=diff[:], in1=psel[:], op=AluOpType.mult)
            bigp = wp.tile([128, AI], f32)
            nc.vector.tensor_scalar(out=bigp[:], in0=psel[:], scalar1=-float(BIG), scalar2=None, op0=AluOpType.mult)
            nc.vector.tensor_scalar(out=bigp[:], in0=bigp[:], scalar1=float(BIG), scalar2=None, op0=AluOpType.add)
            nc.vector.tensor_tensor(out=ds[:], in0=ds[:], in1=bigp[:], op=AluOpType.add)
            ds_i = wp.tile([128, AI], i32)
            nc.vector.tensor_copy(ds_i[:], ds[:].bitcast(i32))
            hb_i = wp.tile([128, AI], i32)
            nc.vector.tensor_scalar(out=hb_i[:], in0=ds_i[:], scalar1=8, scalar2=None, op0=AluOpType.logical_shift_right)
            lb_i = wp.tile([128, AI], i32)
            nc.vector.tensor_scalar(out=lb_i[:], in0=ds_i[:], scalar1=255, scalar2=None, op0=AluOpType.bitwise_and)
            hb = cp.tile([128, AI], f32)
            nc.vector.tensor_copy(hb[:], hb_i[:])
            lb = cp.tile([128, AI], f32)
            nc.vector.tensor_copy(lb[:], lb_i[:])

            # ---------- exact k-select via 16-ary bisection ----------
            iot_i = wp.tile([128, 15], i32)
            nc.gpsimd.iota(iot_i[:], pattern=[[1, 15]], base=1, channel_multiplier=0)
            iot = cp.tile([128, 15], f32)
            nc.vector.tensor_copy(iot[:], iot_i[:])

            with tc.tile_pool(name='ps_sel', bufs=2, space='PSUM') as pss, \
                 tc.tile_pool(name='selw', bufs=2) as sw:

                HUGE = 1.0e9

                def kselect(data_f, kf, hi_init, nrounds, tagn):
                    # pure-f32 16-ary bisection; values stay < 2^24 so all
                    # arithmetic that must be exact (terminal step=1 probes)
                    # is exact; mid-round fractional probes are harmless.
                    lo = sw.tile([128, 1], f32, name=f"lo_{tagn}")
                    hi = sw.tile([128, 1], f32, name=f"hi_{tagn}")
                    nc.vector.memset(lo[:], 0.0)
                    nc.vector.memset(hi[:], float(hi_init))
                    for r in range(nrounds):
                        st = sw.tile([128, 1], f32, name=f"st_{tagn}", tag=f"st{tagn}")
                        nc.vector.tensor_tensor(out=st[:], in0=hi[:], in1=lo[:], op=AluOpType.subtract)
                        nc.vector.tensor_scalar(out=st[:], in0=st[:], scalar1=0.0625, scalar2=1.0, op0=AluOpType.mult, op1=AluOpType.max)
                        pr = sw.tile([128, 15], f32, name=f"pr_{tagn}", tag=f"pr{tagn}")
                        nc.vector.tensor_scalar(out=pr[:], in0=iot[:], scalar1=st[:], scalar2=lo[:], op0=AluOpType.mult, op1=AluOpType.add)
                        cmp = sw.tile([128, 15, AI], f32, name=f"cmp_{tagn}", tag=f"cmp{tagn}")
                        nc.vector.tensor_tensor(out=cmp[:],
                                                in0=data_f[:, None, :].broadcast_to([128, 15, AI]),
                                                in1=pr[:, :, None].broadcast_to([128, 15, AI]),
                                                op=AluOpType.is_lt)
                        pcnt = sw.tile([128, 15], f32, name=f"pc_{tagn}", tag=f"pc{tagn}")
                        nc.vector.tensor_reduce(pcnt[:], cmp[:], axis=AX.X, op=AluOpType.add)
                        ct_ps = pss.tile([128, 15], f32, name=f"ct_{tagn}", tag=f"ct{tagn}")
                        nc.tensor.matmul(ct_ps[:], lhsT=ones[:], rhs=pcnt[:], start=True, stop=True)
                        flag = sw.tile([128, 15], f32, name=f"fl_{tagn}", tag=f"fl{tagn}")
                        nc.vector.tensor_tensor(out=flag[:], in0=ct_ps[:], in1=kf[:].broadcast_to([128, 15]), op=AluOpType.is_ge)
                        fl2 = sw.tile([128, 15], f32, name=f"fl2_{tagn}", tag=f"fl2{tagn}")
                        nc.vector.tensor_scalar(out=fl2[:], in0=flag[:], scalar1=HUGE, scalar2=None, op0=AluOpType.mult)
                        sel = sw.tile([128, 15], f32, name=f"sel_{tagn}", tag=f"sel{tagn}")
                        nc.vector.tensor_tensor(out=sel[:], in0=pr[:], in1=fl2[:], op=AluOpType.subtract)
                        nl = sw.tile([128, 1], f32, name=f"nl_{tagn}", tag=f"nl{tagn}")
                        nc.vector.tensor_reduce(nl[:], sel[:], axis=AX.X, op=AluOpType.max)
                        nc.vector.tensor_tensor(out=lo[:], in0=lo[:], in1=nl[:], op=AluOpType.max)
                        t2 = sw.tile([128, 15], f32, name=f"t2_{tagn}", tag=f"t2{tagn}")
                        nc.vector.tensor_scalar(out=t2[:], in0=fl2[:], scalar1=-1.0, scalar2=HUGE, op0=AluOpType.mult, op1=AluOpType.add)
                        nc.vector.tensor_tensor(out=sel[:], in0=pr[:], in1=t2[:], op=AluOpType.add)
                        nh = sw.tile([128, 1], f32, name=f"nh_{tagn}", tag=f"nh{tagn}")
                        nc.vector.tensor_reduce(nh[:], sel[:], axis=AX.X, op=AluOpType.min)
                        nc.vector.tensor_tensor(out=hi[:], in0=hi[:], in1=nh[:], op=AluOpType.min)
                    return lo

                bstar = kselect(hb, kk_f, HB_HI, 7, "h")          # high-23 bits of thr

                # r1 = count(hb < B*), k2 = k - r1
                cmpb = sw.tile([128, AI], f32)
                nc.vector.tensor_tensor(out=cmpb[:], in0=hb[:], in1=bstar[:].broadcast_to([128, AI]), op=AluOpType.is_lt)
                r1p = sw.tile([128, 1], f32)
                nc.vector.tensor_reduce(r1p[:], cmpb[:], axis=AX.X, op=AluOpType.add)
                r1_ps = pss.tile([128, 1], f32)
                nc.tensor.matmul(r1_ps[:], lhsT=ones[:], rhs=r1p[:], start=True, stop=True)
                k2f = sw.tile([128, 1], f32)
                nc.vector.tensor_tensor(out=k2f[:], in0=kk_f[:], in1=r1_ps[:], op=AluOpType.subtract)
                r1 = sw.tile([128, 1], f32)
                nc.vector.tensor_copy(r1[:], r1_ps[:])

                # cand = lb where hb==B* else 256
                eqb = sw.tile([128, AI], f32)
                nc.vector.tensor_tensor(out=eqb[:], in0=hb[:], in1=bstar[:].broadcast_to([128, AI]), op=AluOpType.is_equal)
                cand = sw.tile([128, AI], f32)
                nc.vector.tensor_tensor(out=cand[:], in0=lb[:], in1=eqb[:], op=AluOpType.mult)
                inv2 = sw.tile([128, AI], f32)
                nc.vector.tensor_scalar(out=inv2[:], in0=eqb[:], scalar1=-256.0, scalar2=256.0, op0=AluOpType.mult, op1=AluOpType.add)
                nc.vector.tensor_tensor(out=cand[:], in0=cand[:], in1=inv2[:], op=AluOpType.add)

                lstar = kselect(cand, k2f, 257, 4, "l")           # low-8 bits of thr

                # keep = (hb < B*) | (cand < L*)   (disjoint)
                keep = sw.tile([128, AI], f32)
                cl = sw.tile([128, AI], f32)
                nc.vector.tensor_tensor(out=cl[:], in0=cand[:], in1=lstar[:].broadcast_to([128, AI]), op=AluOpType.is_lt)
                nc.vector.tensor_tensor(out=keep[:], in0=cmpb[:], in1=cl[:], op=AluOpType.add)

                # ---------- final loss ----------
                mk = sw.tile([128, AI], f32)
                nc.vector.tensor_tensor(out=mk[:], in0=keep[:], in1=mnat[:], op=AluOpType.mult)
                d2 = sw.tile([128, AI], f32)
                nc.vector.tensor_tensor(out=d2[:], in0=diff[:], in1=diff[:], op=AluOpType.mult)
                nc.vector.tensor_tensor(out=d2[:], in0=d2[:], in1=mk[:], op=AluOpType.mult)
                s2 = sw.tile([128, 2], f32)
                nc.vector.tensor_reduce(s2[:, 0:1], d2[:], axis=AX.X, op=AluOpType.add)
                nc.vector.tensor_reduce(s2[:, 1:2], mk[:], axis=AX.X, op=AluOpType.add)
                s2_ps = pss.tile([128, 2], f32)
                nc.tensor.matmul(s2_ps[:], lhsT=ones[:], rhs=s2[:], start=True, stop=True)
                s2a = sw.tile([128, 2], f32)
                nc.vector.tensor_copy(s2a[:], s2_ps[:])
                den = sw.tile([128, 1], f32)
                nc.vector.tensor_scalar(out=den[:], in0=s2a[:, 1:2], scalar1=1e-12, scalar2=None, op0=AluOpType.add)
                rden = sw.tile([128, 1], f32)
                nc.vector.reciprocal(rden[:], den[:])
                lb_t = sw.tile([128, 1], f32)
                nc.vector.tensor_tensor(out=lb_t[:], in0=s2a[:, 0:1], in1=rden[:], op=AluOpType.mult)
                nc.vector.tensor_scalar(out=lb_t[:], in0=lb_t[:], scalar1=0.125, scalar2=None, op0=AluOpType.mult)

                # global mean over batches: AllReduce(add) of loss_b/8 over 8 cores
                cc2i = dp.tile([1, 1], f32)
                cc2o = dp.tile([1, 1], f32)
                nc.sync.dma_start(cc2i[:], lb_t[0:1, 0:1])
                nc.gpsimd.collective_compute(
                    "AllReduce", AluOpType.add,
                    replica_groups=[[0, 1, 2, 3, 4, 5, 6, 7]],
                    ins=[cc2i[:]], outs=[cc2o[:]])
                lossg = sw.tile([1, 1], f32)
                nc.sync.dma_start(lossg[:], cc2o[:])

                # out = exp(-alpha) * loss + alpha
                ea = sw.tile([1, 1], f32)
                nc.scalar.activation(ea[:], alph[:], AF.Exp, scale=-1.0)
                ov = sw.tile([1, 1], f32)
                nc.vector.tensor_tensor(out=ov[:], in0=ea[:], in1=lossg[:], op=AluOpType.mult)
                nc.vector.tensor_tensor(out=ov[:], in0=ov[:], in1=alph[:], op=AluOpType.add)
                nc.sync.dma_start(out_d[:], ov[:])

                # debug row: n_ip, n_it, n_sel, k, B*, L*, r1, loss_b*8... (per-partition col dump)
                dbgt = sw.tile([128, 8], f32)
                nc.vector.tensor_copy(dbgt[:, 0:1], c2a[:, 0:1])
                nc.vector.tensor_copy(dbgt[:, 1:2], c2a[:, 1:2])
                nc.vector.tensor_copy(dbgt[:, 2:3], nsa[:])
                nc.vector.tensor_copy(dbgt[:, 3:4], kk_f[:])
                nc.vector.tensor_copy(dbgt[:, 4:5], bstar[:])
                nc.vector.tensor_copy(dbgt[:, 5:6], lstar[:])
                nc.vector.tensor_copy(dbgt[:, 6:7], r1[:])
                nc.vector.tensor_copy(dbgt[:, 7:8], lb_t[:])
                nc.sync.dma_start(dbg_d[:], dbgt[:])

    return nc


# --------------------------------------------------------------------------
# host wrapper
# --------------------------------------------------------------------------
_NC_CACHE = {}


def _get_nc():
    if 'nc' not in _NC_CACHE:
        _NC_CACHE['nc'] = build_nc()
    return _NC_CACHE['nc']


def _marshal(prediction_tensor, target_tensor, mask, alpha):
    pred = np.asarray(prediction_tensor, np.float32)
    tgt = np.asarray(target_tensor, np.float32)
    msk = np.asarray(mask, np.float32)
    al = np.asarray(alpha, np.float32).reshape(1, 1)

    in_maps = []
    for c in range(N_CORES):
        b, h = c // 2, c % 2
        p = np.empty((NI, 3), np.float32)
        p[:N] = pred[b]
        p[N:] = pred[b, 0]
        t = np.full((NI, 3), PADV, np.float32)
        t[:N] = tgt[b]
        th = np.full((NJ, 3), PADV, np.float32)
        th[:MH] = tgt[b, h * MH:(h + 1) * MH]
        m = np.zeros(NI, np.float32)
        m[:N] = msk[b]
        in_maps.append({
            'pred_pm': np.ascontiguousarray(p.reshape(128, AI * 3)),
            'pred_nat': np.ascontiguousarray(
                p.reshape(AI, 128, 3).transpose(1, 0, 2).reshape(128, AI * 3)),
            'tgt_nat': np.ascontiguousarray(
                t.reshape(AI, 128, 3).transpose(1, 0, 2).reshape(128, AI * 3)),
            'tgt_half_pm': np.ascontiguousarray(th.reshape(128, AJ * 3)),
            'mask_nat': np.ascontiguousarray(m.reshape(AI, 128).T),
            'valid_nat': np.ascontiguousarray(
                (np.arange(NI) < N).astype(np.float32).reshape(AI, 128).T),
            'alpha_in': al,
        })
    return in_maps


def run_cores(prediction_tensor, target_tensor, mask, alpha, **rb_kwargs):
    nc = _get_nc()
    in_maps = _marshal(prediction_tensor, target_tensor, mask, alpha)
    return run_bass_kernel_spmd(nc, in_maps, core_ids=list(range(N_CORES)), **rb_kwargs)


def kernel(prediction_tensor, target_tensor, mask, alpha):
    res = run_cores(prediction_tensor, target_tensor, mask, alpha)
    return res.results[0]['out'].reshape(1).astype(np.float32)


# revision 24
# speedup vs baseline: 1.1620x; 1.0459x over previous
"""Chamfer L2 loss (nn_ChamferL2Loss) Trainium2 Bass kernel.

Strategy: 8 NeuronCores, core c handles batch b=c//2 and target-half h=c%2.
Each core computes row-mins of the [7000 x 3500] squared-distance matrix for
its half via K=4 matmuls (coords + fused |t|^2 + column-mask row), DVE
reduce-min from PSUM, then an AllReduce(min) within core pairs merges halves.
The kth-value threshold (jnp.sort + take in the reference) is found with an
exact bit-pattern multi-way bisection (high-23/low-8 bit split keeps all DVE
integer arithmetic within fp32-exact range).  Final per-batch losses are
averaged with an 8-core AllReduce(add).
"""

import numpy as np

import concourse.bass as bass
import concourse.tile as tile
import concourse.mybir as mybir
from concourse.alu_op_type import AluOpType
from concourse.vector_clock import ScopedClock
from concourse.bass_utils import run_bass_kernel_spmd

f32 = mybir.dt.float32
bf16 = mybir.dt.bfloat16
i32 = mybir.dt.int32
fp16 = mybir.dt.float16
AX = mybir.AxisListType
AF = mybir.ActivationFunctionType

B = 4
N = 7000          # points per cloud
NI = 7040         # padded rows (55 * 128)
AI = 55           # NI / 128
MH = 3500         # targets per core (half)
NJ = 3584         # padded cols (28 * 128 = 8 * 448)
AJ = 28           # NJ / 128
JT = 448          # matmul free-dim tile
BIG = np.float32(1e10)
PADV = np.float32(1e4)
MARGIN = 0.05
MIN_PTS = 500.0
HB_HI = 8388609   # 2^23 + 1: exclusive upper bound for high-23-bit patterns

N_CORES = 8


# --------------------------------------------------------------------------
# Custom DVE op: out = min(in0, in1); accum_out = min(C0, min_k out[k]).
# Consumes two tiles per instruction (both DVE read ports), halving the
# per-element cost of the row-min versus tensor_reduce.  Registered via the
# documented extension point in concourse.dve_ops (define + append to OPS).
# --------------------------------------------------------------------------
def _register_minmin():
    from concourse import dve_ops
    from concourse.dve_spec import Spec, Src0, Src1, C0, minn
    name = "TT_MIN_REDUCE_ANT"
    for o in dve_ops.OPS:
        if o.name == name:
            return o
    op = dve_ops.DveOp(
        name,
        Spec(body=minn(Src0, Src1), accum=minn, accum_init=C0,
             reference=lambda in0, in1, c0, c1, c2: np.minimum(
                 in0.astype(np.float32), in1.astype(np.float32))),
        subdim=False,
        uops_sha={"v3": "80668f319ac378ba", "v4": "23f6c1536de15f6a"},
    )
    dve_ops.OPS.append(op)
    dve_ops.CUSTOM_DVE_SPECS[name] = op.spec
    dve_ops._SUB_OPCODE_FOR_NAME[name] = max(dve_ops._SUB_OPCODE_FOR_NAME.values()) + 1
    assert dve_ops._SUB_OPCODE_FOR_NAME[name] < 0x20
    return op


MINMIN = _register_minmin()


# --------------------------------------------------------------------------
# TileContext workaround: this container's walrus build rejects instructions
# carrying more than one semaphore wait ("Too many sync wait commands").
# Split extra waits onto single-wait NOPs inserted just before the holder.
# --------------------------------------------------------------------------
def _split_multiwaits(nc, max_waits=1):
    for f in nc.m.functions:
        for bb in f.blocks:
            insts = bb.instructions
            idx = 0
            while idx < len(insts):
                inst = insts[idx]
                si = inst.sync_info
                if si is not None and len(si.on_wait) > max_waits:
                    waits = list(si.on_wait)
                    inst.sync_info = mybir.SyncInfo(
                        on_wait=waits[:max_waits], on_update=list(si.on_update))
                    for w in waits[max_waits:]:
                        nop = mybir.InstNoOp(
                            name=f"waitsplit-{nc.next_id()}", ins=[], outs=[])
                        nop.engine = inst.engine
                        nop.sync_info = mybir.SyncInfo(on_wait=[w], on_update=[])
                        nc.register_instruction(nop)
                        insts.insert(idx, nop)
                        idx += 1
                idx += 1


class TC(tile.TileContext):
    def schedule_and_allocate(self, validate_deps=False):
        r = super().schedule_and_allocate(validate_deps=validate_deps)
        _split_multiwaits(self.nc)
        return r


# --------------------------------------------------------------------------
# device program
# --------------------------------------------------------------------------
def _ptree_fold32(nc, pool, src, op):
    """Reduce [128, F] across partitions to [32, F] via 2 pairwise folds
    (engine SBUF accesses must start at 32-aligned partitions)."""
    f = src.shape[-1]
    h64 = pool.tile([64, f], f32, name=f"foldc64_{nc.next_id()}")
    nc.vector.tensor_copy(h64[:], src[64:128, :])
    t64 = pool.tile([64, f], f32, name=f"fold64_{nc.next_id()}")
    nc.vector.tensor_tensor(out=t64[:], in0=src[0:64, :], in1=h64[:], op=op)
    h32 = pool.tile([32, f], f32, name=f"foldc32_{nc.next_id()}")
    nc.vector.tensor_copy(h32[:], t64[32:64, :])
    t32 = pool.tile([32, f], f32, name=f"fold32_{nc.next_id()}")
    nc.vector.tensor_tensor(out=t32[:], in0=t64[0:32, :], in1=h32[:], op=op)
    return t32


def build_nc():
    nc = bass.Bass(num_devices=N_CORES)

    pred_pm = nc.declare_dram_parameter('pred_pm', [128, AI * 3], f32, isOutput=False)
    pred_nat = nc.declare_dram_parameter('pred_nat', [128, AI * 3], f32, isOutput=False)
    tgt_nat = nc.declare_dram_parameter('tgt_nat', [128, AI * 3], f32, isOutput=False)
    tgt_half_pm = nc.declare_dram_parameter('tgt_half_pm', [128, AJ * 3], f32, isOutput=False)
    mask_nat = nc.declare_dram_parameter('mask_nat', [128, AI], f32, isOutput=False)
    valid_nat = nc.declare_dram_parameter('valid_nat', [128, AI], f32, isOutput=False)
    alpha_in = nc.declare_dram_parameter('alpha_in', [1, 1], f32, isOutput=False)

    out_d = nc.declare_dram_parameter('out', [1, 1], f32, isOutput=True)
    dbg_d = nc.declare_dram_parameter('dbg', [128, 8], f32, isOutput=True)
    dbg2_d = nc.declare_dram_parameter('dbg2', [128, 8], f32, isOutput=True)
    dbg_diff = nc.declare_dram_parameter('dbg_diff', [128, AI], f32, isOutput=True)

    with TC(nc) as tc:
        with tc.tile_pool(name='const', bufs=1) as cp, \
             tc.tile_pool(name='work', bufs=2) as wp, \
             tc.tile_pool(name='dram', bufs=1, space='DRAM') as dp:

            # ---------- loads ----------
            # bf16 split-precision matmul, K=21:
            #   lhsT rows: P1 P1 P1 P2 P2 P3 (x3 coords) + three ones rows
            #   rhs rows:  V1 V2 V3 V1 V2 V1 (x3 coords, V=-2t) + w1 w2 w3
            # where X = X1+X2+X3 is a 3-term bf16 split and w is the 3-term
            # split of |t|^2 + (1-tsel)*BIG.  Dropped cross terms are
            # O(|p||t| 2^-26).  Rows are assembled via a DRAM staging
            # buffer (engine writes must start at 32-aligned partitions,
            # DMA round-trip through DRAM sidesteps that).
            ppm = cp.tile([128, AI * 3], f32)
            nc.sync.dma_start(ppm[:], pred_pm[:])

            pnat = cp.tile([128, AI * 3], f32)
            nc.sync.dma_start(pnat[:], pred_nat[:])
            tnat = cp.tile([128, AI * 3], f32)
            nc.sync.dma_start(tnat[:], tgt_nat[:])
            thpm = cp.tile([128, AJ * 3], f32)
            nc.scalar.dma_start(thpm[:], tgt_half_pm[:])
            mnat = cp.tile([128, AI], f32)
            nc.scalar.dma_start(mnat[:], mask_nat[:])
            vnat = cp.tile([128, AI], f32)
            nc.scalar.dma_start(vnat[:], valid_nat[:])
            alph = cp.tile([1, 1], f32)
            nc.sync.dma_start(alph[:], alpha_in[:])

            ones = cp.tile([128, 128], f32)
            nc.vector.memset(ones[:], 1.0)

            pnat3 = pnat[:].rearrange("p (a k) -> p a k", k=3)
            tnat3 = tnat[:].rearrange("p (a k) -> p a k", k=3)
            thpm3 = thpm[:].rearrange("p (a k) -> p a k", k=3)

            ppm3 = ppm[:].rearrange("p (a k) -> p a k", k=3)

            stage_l = dp.tile([21, NI], f32)
            stage_r = dp.tile([21, NJ], f32)
            onesAI = wp.tile([128, AI], f32)
            nc.vector.memset(onesAI[:], 1.0)

            def cdu(dst, src_ap, cols, tagn):
                # dst <- f32(bf16(src)): round-trip through bf16
                tmpb = wp.tile([128, cols], bf16, name=f"cdub_{nc.next_id()}", tag=f"cdub{tagn}")
                nc.vector.tensor_copy(tmpb[:], src_ap)
                nc.vector.tensor_copy(dst[:], tmpb[:])

            def split3(src_ap, cols, tagn):
                s1 = wp.tile([128, cols], f32, name=f"s1_{nc.next_id()}", tag=f"s1{tagn}")
                s2 = wp.tile([128, cols], f32, name=f"s2_{nc.next_id()}", tag=f"s2{tagn}")
                s3 = wp.tile([128, cols], f32, name=f"s3_{nc.next_id()}", tag=f"s3{tagn}")
                r = wp.tile([128, cols], f32, name=f"r_{nc.next_id()}", tag=f"r{tagn}")
                cdu(s1, src_ap, cols, tagn)
                nc.vector.tensor_tensor(out=r[:], in0=src_ap, in1=s1[:], op=AluOpType.subtract)
                cdu(s2, r[:], cols, tagn)
                nc.vector.tensor_tensor(out=r[:], in0=r[:], in1=s2[:], op=AluOpType.subtract)
                cdu(s3, r[:], cols, tagn)
                return s1, s2, s3

            # lhsT planes: pred splits (pm layout, point = p*AI + a)
            for k in range(3):
                p1, p2, p3 = split3(ppm3[:, :, k], AI, "p")
                for row, t in ((0, p1), (3, p1), (6, p1), (9, p2), (12, p2), (15, p3)):
                    nc.scalar.dma_start(stage_l[row + k:row + k + 1, :], t[:])
            for row in (18, 19, 20):
                nc.scalar.dma_start(stage_l[row:row + 1, :], onesAI[:])

            # rhs coordinate planes: V = -2*t splits (pm layout, point = p*AJ + a)
            for k in range(3):
                vneg = wp.tile([128, AJ], f32, name=f"vneg_{k}", tag="vneg")
                nc.vector.tensor_scalar(out=vneg[:], in0=thpm3[:, :, k], scalar1=-2.0, scalar2=None, op0=AluOpType.mult)
                t1, t2, t3 = split3(vneg[:], AJ, "t")
                for row, t in ((0, t1), (3, t2), (6, t3), (9, t1), (12, t2), (15, t1)):
                    nc.sync.dma_start(stage_r[row + k:row + k + 1, :], t[:])

            lhsT_f = cp.tile([21, NI], f32)
            nc.scalar.dma_start(lhsT_f[:], stage_l[:])
            lhsT_bf = cp.tile([21, NI], bf16)
            nc.vector.tensor_copy(lhsT_bf[:], lhsT_f[:])

            # ---------- |p|^2 per point (nat layout), |t|^2 per half target (pm layout)
            sqp = wp.tile([128, AI * 3], f32)
            nc.vector.tensor_tensor(out=sqp[:], in0=pnat[:], in1=pnat[:], op=AluOpType.mult)
            sqp3 = sqp[:].rearrange("p (a k) -> p a k", k=3)
            pp = cp.tile([128, AI], f32)
            nc.vector.tensor_tensor(out=pp[:], in0=sqp3[:, :, 0], in1=sqp3[:, :, 1], op=AluOpType.add)
            nc.vector.tensor_tensor(out=pp[:], in0=pp[:], in1=sqp3[:, :, 2], op=AluOpType.add)

            sqt = wp.tile([128, AJ * 3], f32)
            nc.vector.tensor_tensor(out=sqt[:], in0=thpm[:], in1=thpm[:], op=AluOpType.mult)
            sqt3 = sqt[:].rearrange("p (a k) -> p a k", k=3)
            ttpm = cp.tile([128, AJ], f32)
            nc.vector.tensor_tensor(out=ttpm[:], in0=sqt3[:, :, 0], in1=sqt3[:, :, 1], op=AluOpType.add)
            nc.vector.tensor_tensor(out=ttpm[:], in0=ttpm[:], in1=sqt3[:, :, 2], op=AluOpType.add)

            # ---------- bounds from pred (exact min/max over the 7000 real rows)
            # pad rows hold PADV=1e4 > any real coord, fine for max; for min
            # they lose to real values (real coords ~N(0,1), all < 1e4).
            # PADV would corrupt min only if all values padded - not the case.
            mx32 = _ptree_fold32(nc, wp, pnat[:], AluOpType.max)   # [32, 165]
            mn32 = _ptree_fold32(nc, wp, pnat[:], AluOpType.min)   # [32, 165]
            mxc = wp.tile([32, 3], f32)
            mnc = wp.tile([32, 3], f32)
            mx32v = mx32[:].rearrange("p (a k) -> p k a", k=3)
            mn32v = mn32[:].rearrange("p (a k) -> p k a", k=3)
            nc.vector.tensor_reduce(mxc[:], mx32v, axis=AX.X, op=AluOpType.max)
            nc.vector.tensor_reduce(mnc[:], mn32v, axis=AX.X, op=AluOpType.min)
            mxf = wp.tile([1, 96], f32)
            mnf = wp.tile([1, 96], f32)
            nc.scalar.dma_start(mxf[:], mxc[:])
            nc.scalar.dma_start(mnf[:], mnc[:])
            mx13 = wp.tile([1, 3], f32)
            mn13 = wp.tile([1, 3], f32)
            nc.vector.tensor_reduce(mx13[:], mxf[:].rearrange("o (g k) -> o k g", k=3), axis=AX.X, op=AluOpType.max)
            nc.vector.tensor_reduce(mn13[:], mnf[:].rearrange("o (g k) -> o k g", k=3), axis=AX.X, op=AluOpType.min)

            # lo = mn + 0.05*w ; hi = mx - 0.05*w ; w = mx - mn     (f32, as ref)
            w13 = wp.tile([1, 3], f32)
            nc.vector.tensor_tensor(out=w13[:], in0=mx13[:], in1=mn13[:], op=AluOpType.subtract)
            mw = wp.tile([1, 3], f32)
            nc.vector.tensor_scalar(out=mw[:], in0=w13[:], scalar1=float(MARGIN), scalar2=None, op0=AluOpType.mult)
            lo13 = wp.tile([1, 3], f32)
            nc.vector.tensor_tensor(out=lo13[:], in0=mn13[:], in1=mw[:], op=AluOpType.add)
            hi13 = wp.tile([1, 3], f32)
            nc.vector.tensor_tensor(out=hi13[:], in0=mx13[:], in1=mw[:], op=AluOpType.subtract)
            hl13 = wp.tile([1, 3], f32)
            nc.vector.tensor_tensor(out=hl13[:], in0=hi13[:], in1=lo13[:], op=AluOpType.subtract)
            # r_lo = (hi-lo)*bi*bs + lo ; r_hi = r_lo + (hi-lo)*bs
            bibs = wp.tile([1, 3], f32)   # bi*bs = [0.4, 0, 0]
            nc.vector.memset(bibs[:], 0.0)
            nc.vector.memset(bibs[0:1, 0:1], 0.4)
            bs13 = wp.tile([1, 3], f32)   # bs = [0.1, 1, 1]
            nc.vector.memset(bs13[:], 1.0)
            nc.vector.memset(bs13[0:1, 0:1], 0.1)
            t13 = wp.tile([1, 3], f32)
            nc.vector.tensor_tensor(out=t13[:], in0=hl13[:], in1=bibs[:], op=AluOpType.mult)
            rlo13 = wp.tile([1, 6], f32)
            nc.vector.tensor_tensor(out=rlo13[:, 0:3], in0=t13[:], in1=lo13[:], op=AluOpType.add)
            nc.vector.tensor_tensor(out=t13[:], in0=hl13[:], in1=bs13[:], op=AluOpType.mult)
            nc.vector.tensor_tensor(out=rlo13[:, 3:6], in0=rlo13[:, 0:3], in1=t13[:], op=AluOpType.add)

            # broadcast [1,6] -> [128,6] via K=1 matmul with ones
            with tc.tile_pool(name='ps_pre', bufs=1, space='PSUM') as psp:
                rl_ps = psp.tile([128, 6], f32)
                nc.tensor.matmul(rl_ps[:], lhsT=ones[0:1, :], rhs=rlo13[:], start=True, stop=True)
                rlh = cp.tile([128, 6], f32)
                nc.vector.tensor_copy(rlh[:], rl_ps[:])

                # ---------- indicators (strict > r_lo and < r_hi on all 3 dims)
                def indicator(dst, src3, acols):
                    tmp = wp.tile([128, acols], f32, name=f"indt_{nc.next_id()}", tag="indt")
                    for k in range(3):
                        nc.vector.tensor_scalar(out=(dst if k == 0 else tmp)[:, 0:acols], in0=src3[:, :, k],
                                                scalar1=rlh[:, k:k + 1], scalar2=None, op0=AluOpType.is_gt)
                        if k > 0:
                            nc.vector.tensor_tensor(out=dst[:, 0:acols], in0=dst[:, 0:acols], in1=tmp[:, 0:acols], op=AluOpType.mult)
                        nc.vector.tensor_scalar(out=tmp[:, 0:acols], in0=src3[:, :, k],
                                                scalar1=rlh[:, 3 + k:4 + k], scalar2=None, op0=AluOpType.is_lt)
                        nc.vector.tensor_tensor(out=dst[:, 0:acols], in0=dst[:, 0:acols], in1=tmp[:, 0:acols], op=AluOpType.mult)

                ip = cp.tile([128, AI], f32)
                indicator(ip, pnat3, AI)
                # pred_nat pads replicate point 0 (keeps bounds exact), so
                # mask pad rows out of the indicator explicitly
                nc.vector.tensor_tensor(out=ip[:], in0=ip[:], in1=vnat[:], op=AluOpType.mult)
                itf = wp.tile([128, AI], f32)
                indicator(itf, tnat3, AI)
                ith = cp.tile([128, AJ], f32)
                indicator(ith, thpm3, AJ)

                dbg2t = wp.tile([128, 8], f32)
                nc.vector.tensor_copy(dbg2t[:, 0:6], rlh[:])
                nc.vector.tensor_reduce(dbg2t[:, 6:7], ip[:], axis=AX.X, op=AluOpType.add)
                nc.vector.tensor_reduce(dbg2t[:, 7:8], pnat3[:, :, 0], axis=AX.X, op=AluOpType.max)
                nc.sync.dma_start(dbg2_d[:], dbg2t[:])

                # counts over full clouds (pads indicate 0)
                c2 = wp.tile([128, 2], f32)
                nc.vector.tensor_reduce(c2[:, 0:1], ip[:], axis=AX.X, op=AluOpType.add)
                nc.vector.tensor_reduce(c2[:, 1:2], itf[:], axis=AX.X, op=AluOpType.add)
                c2_ps = psp.tile([128, 2], f32)
                nc.tensor.matmul(c2_ps[:], lhsT=ones[:], rhs=c2[:], start=True, stop=True)
                c2a = cp.tile([128, 2], f32)
                nc.vector.tensor_copy(c2a[:], c2_ps[:])

                # psel = ip if n_ip >= 500 else onehot0
                flagp = cp.tile([128, 1], f32)
                nc.vector.tensor_scalar(out=flagp[:], in0=c2a[:, 0:1], scalar1=MIN_PTS, scalar2=None, op0=AluOpType.is_ge)
                invp = cp.tile([128, 1], f32)
                nc.vector.tensor_scalar(out=invp[:], in0=flagp[:], scalar1=-1.0, scalar2=None, op0=AluOpType.mult)
                nc.vector.tensor_scalar(out=invp[:], in0=invp[:], scalar1=1.0, scalar2=None, op0=AluOpType.add)
                psel = cp.tile([128, AI], f32)
                nc.vector.tensor_scalar(out=psel[:], in0=ip[:], scalar1=flagp[:], scalar2=None, op0=AluOpType.mult)
                oneh = wp.tile([128, AI], f32)
                nc.vector.memset(oneh[:], 0.0)
                nc.vector.memset(oneh[0:1, 0:1], 1.0)
                nc.vector.tensor_scalar(out=oneh[:], in0=oneh[:], scalar1=invp[:], scalar2=None, op0=AluOpType.mult)
                nc.vector.tensor_tensor(out=psel[:], in0=psel[:], in1=oneh[:], op=AluOpType.add)

                # tsel_half = ith if n_it >= 500 else ones
                flagt = cp.tile([128, 1], f32)
                nc.vector.tensor_scalar(out=flagt[:], in0=c2a[:, 1:2], scalar1=MIN_PTS, scalar2=None, op0=AluOpType.is_ge)
                invt = cp.tile([128, 1], f32)
                nc.vector.tensor_scalar(out=invt[:], in0=flagt[:], scalar1=-1.0, scalar2=None, op0=AluOpType.mult)
                nc.vector.tensor_scalar(out=invt[:], in0=invt[:], scalar1=1.0, scalar2=None, op0=AluOpType.add)
                tsel = wp.tile([128, AJ], f32)
                nc.vector.tensor_scalar(out=tsel[:], in0=ith[:], scalar1=flagt[:], scalar2=None, op0=AluOpType.mult)
                nc.vector.tensor_scalar(out=tsel[:], in0=tsel[:], scalar1=invt[:], scalar2=None, op0=AluOpType.add)

                # combined rhs row: |t|^2 + (1-tsel)*BIG   (pm layout)
                cmb = cp.tile([128, AJ], f32)
                nc.vector.tensor_scalar(out=cmb[:], in0=tsel[:], scalar1=-float(BIG), scalar2=None, op0=AluOpType.mult)
                nc.vector.tensor_scalar(out=cmb[:], in0=cmb[:], scalar1=float(BIG), scalar2=None, op0=AluOpType.add)
                nc.vector.tensor_tensor(out=cmb[:], in0=cmb[:], in1=ttpm[:], op=AluOpType.add)
                # 3-term bf16 split of |t|^2+mask -> staging rows 18-20
                w1, w2, w3 = split3(cmb[:], AJ, "w")
                nc.sync.dma_start(stage_r[18:19, :], w1[:])
                nc.sync.dma_start(stage_r[19:20, :], w2[:])
                nc.sync.dma_start(stage_r[20:21, :], w3[:])
                rhs_f = cp.tile([21, NJ], f32)
                nc.sync.dma_start(rhs_f[:], stage_r[:])
                rhs_bf = cp.tile([21, NJ], bf16)
                nc.vector.tensor_copy(rhs_bf[:], rhs_f[:])

                # n_sel and threshold index k = 1 + (n_sel >> 1)
                nsp = wp.tile([128, 1], f32)
                nc.vector.tensor_reduce(nsp[:], psel[:], axis=AX.X, op=AluOpType.add)
                ns_ps = psp.tile([128, 1], f32)
                nc.tensor.matmul(ns_ps[:], lhsT=ones[:], rhs=nsp[:], start=True, stop=True)
                nsa = cp.tile([128, 1], f32)
                nc.vector.tensor_copy(nsa[:], ns_ps[:])
                ns_i = wp.tile([128, 1], i32)
                nc.vector.tensor_copy(ns_i[:], nsa[:])
                kk_i = cp.tile([128, 1], i32)
                nc.vector.tensor_scalar(out=kk_i[:], in0=ns_i[:], scalar1=1, scalar2=None, op0=AluOpType.logical_shift_right)
                nc.vector.tensor_scalar(out=kk_i[:], in0=kk_i[:], scalar1=1, scalar2=None, op0=AluOpType.add)
                kk_f = cp.tile([128, 1], f32)
                nc.vector.tensor_copy(kk_f[:], kk_i[:])

            # ---------- main loop: 55 i-tiles x 8 matmuls(N=448) ----------
            # Unit u0 (2 banks) is reduced directly from PSUM in fp32; units
            # u1-u3 are converted PSUM->SBUF fp16 by ScalarE, then folded by
            # DVE tensor_tensor min in the 2x packed mode (min of fp16s is
            # exact - it picks one input - only the initial convert rounds).
            pm2 = cp.tile([128, AI, 2], f32)
            diff0 = wp.tile([128, AI], f32)
            CHUNKS = ((0, 28), (28, AI))
            cc1i = [dp.tile([128, c1 - c0], f32, name=f"cc1i{i}") for i, (c0, c1) in enumerate(CHUNKS)]
            cc1o = [dp.tile([128, c1 - c0], f32, name=f"cc1o{i}") for i, (c0, c1) in enumerate(CHUNKS)]
            with tc.tile_pool(name='ps_main', bufs=4, space='PSUM') as psm, \
                 tc.tile_pool(name='cvp', bufs=3) as cvp:
                for it in range(AI):
                    lT = lhsT_bf[:, it * 128:(it + 1) * 128]
                    units = []
                    for u in range(4):
                        pst = psm.tile([128, 2, 512], f32, tag="mm")
                        for s in range(2):
                            j0 = (u * 2 + s) * JT
                            nc.tensor.matmul(pst[:, s, 0:JT], lhsT=lT, rhs=rhs_bf[:, j0:j0 + JT], start=True, stop=True)
                        units.append(pst)
                    nc.vector.tensor_reduce(pm2[:, it, 0:1], units[0][:, :, 0:JT], axis=AX.XY, op=AluOpType.min)
                    # convert with bias=|p|^2 so fp16 rounds the SMALL final
                    # distances, not the large partial (-2pt + tt) values
                    cv = cvp.tile([128, 6 * JT], fp16, tag="cv")
                    for u in (1, 2, 3):
                        nc.scalar.activation(cv[:, (u - 1) * 2 * JT:u * 2 * JT], units[u][:, :, 0:JT],
                                             AF.Identity, bias=pp[:, it:it + 1], scale=1.0)
                    f1 = cvp.tile([128, 3 * JT], fp16, tag="f1")
                    nc.vector.tensor_tensor(out=f1[:], in0=cv[:, 0:3 * JT], in1=cv[:, 3 * JT:6 * JT], op=AluOpType.min)
                    f2 = cvp.tile([128, 3 * JT // 2], fp16, tag="f2")
                    nc.vector.tensor_tensor(out=f2[:], in0=f1[:, 0:3 * JT // 2], in1=f1[:, 3 * JT // 2:3 * JT], op=AluOpType.min)
                    nc.vector.tensor_reduce(pm2[:, it, 1:2], f2[:], axis=AX.X, op=AluOpType.min)

                    # fire the first half of the pair AllReduce as soon as the
                    # first chunk of i-tiles is finished (overlaps main loop)
                    for ci, (c0, c1) in enumerate(CHUNKS):
                        if it == c1 - 1:
                            # col0 mins lack |p|^2 (fp32-direct); col1 already has it
                            pmc = wp.tile([128, c1 - c0], f32, name=f"pmc{ci}", tag="pmc")
                            nc.vector.tensor_tensor(out=pmc[:], in0=pm2[:, c0:c1, 0], in1=pp[:, c0:c1], op=AluOpType.add)
                            nc.vector.tensor_tensor(out=diff0[:, c0:c1], in0=pmc[:], in1=pm2[:, c0:c1, 1], op=AluOpType.min)
                            nc.vector.tensor_scalar(out=diff0[:, c0:c1], in0=diff0[:, c0:c1], scalar1=0.0, scalar2=None, op0=AluOpType.max)
                            nc.sync.dma_start(cc1i[ci][:], diff0[:, c0:c1])
                            nc.gpsimd.collective_compute(
                                "AllReduce", AluOpType.min,
                                replica_groups=[[0, 1], [2, 3], [4, 5], [6, 7]],
                                ins=[cc1i[ci][:]], outs=[cc1o[ci][:]])

            diff = cp.tile([128, AI], f32)
            for ci, (c0, c1) in enumerate(CHUNKS):
                nc.sync.dma_start(diff[:, c0:c1], cc1o[ci][:])
            nc.sync.dma_start(dbg_diff[:], diff[:])

            # ---------- diff_s bits, split high-23 / low-8 ----------
            ds = wp.tile([128, AI], f32)
            nc.vector.tensor_tensor(out=ds[:], in0=diff[:], in1=psel[:], op=AluOpType.mult)
            bigp = wp.tile([128, AI], f32)
            nc.vector.tensor_scalar(out=bigp[:], in0=psel[:], scalar1=-float(BIG), scalar2=None, op0=AluOpType.mult)
            nc.vector.tensor_scalar(out=bigp[:], in0=bigp[:], scalar1=float(BIG), scalar2=None, op0=AluOpType.add)
            nc.vector.tensor_tensor(out=ds[:], in0=ds[:], in1=bigp[:], op=AluOpType.add)
            ds_i = wp.tile([128, AI], i32)
            nc.vector.tensor_copy(ds_i[:], ds[:].bitcast(i32))
            hb_i = wp.tile([128, AI], i32)
            nc.vector.tensor_scalar(out=hb_i[:], in0=ds_i[:], scalar1=8, scalar2=None, op0=AluOpType.logical_shift_right)
            lb_i = wp.tile([128, AI], i32)
            nc.vector.tensor_scalar(out=lb_i[:], in0=ds_i[:], scalar1=255, scalar2=None, op0=AluOpType.bitwise_and)
            hb = cp.tile([128, AI], f32)
            nc.vector.tensor_copy(hb[:], hb_i[:])
            lb = cp.tile([128, AI], f32)
            nc.vector.tensor_copy(lb[:], lb_i[:])

            # ---------- exact k-select via 16-ary bisection ----------
            iot_i = wp.tile([128, 15], i32)
            nc.gpsimd.iota(iot_i[:], pattern=[[1, 15]], base=1, channel_multiplier=0)
            iot = cp.tile([128, 15], f32)
            nc.vector.tensor_copy(iot[:], iot_i[:])

            with tc.tile_pool(name='ps_sel', bufs=2, space='PSUM') as pss, \
                 tc.tile_pool(name='selw', bufs=2) as sw:

                HUGE = 1.0e9

                def kselect(data_f, kf, hi_init, nrounds, tagn):
                    # pure-f32 16-ary bisection; values stay < 2^24 so all
                    # arithmetic that must be exact (terminal step=1 probes)
                    # is exact; mid-round fractional probes are harmless.
                    lo = sw.tile([128, 1], f32, name=f"lo_{tagn}")
                    hi = sw.tile([128, 1], f32, name=f"hi_{tagn}")
                    nc.vector.memset(lo[:], 0.0)
                    nc.vector.memset(hi[:], float(hi_init))
                    for r in range(nrounds):
                        st = sw.tile([128, 1], f32, name=f"st_{tagn}", tag=f"st{tagn}")
                        nc.vector.tensor_tensor(out=st[:], in0=hi[:], in1=lo[:], op=AluOpType.subtract)
                        nc.vector.tensor_scalar(out=st[:], in0=st[:], scalar1=0.0625, scalar2=1.0, op0=AluOpType.mult, op1=AluOpType.max)
                        pr = sw.tile([128, 15], f32, name=f"pr_{tagn}", tag=f"pr{tagn}")
                        nc.vector.tensor_scalar(out=pr[:], in0=iot[:], scalar1=st[:], scalar2=lo[:], op0=AluOpType.mult, op1=AluOpType.add)
                        cmp = sw.tile([128, 15, AI], f32, name=f"cmp_{tagn}", tag=f"cmp{tagn}")
                        nc.vector.tensor_tensor(out=cmp[:],
                                                in0=data_f[:, None, :].broadcast_to([128, 15, AI]),
                                                in1=pr[:, :, None].broadcast_to([128, 15, AI]),
                                                op=AluOpType.is_lt)
                        pcnt = sw.tile([128, 15], f32, name=f"pc_{tagn}", tag=f"pc{tagn}")
                        nc.vector.tensor_reduce(pcnt[:], cmp[:], axis=AX.X, op=AluOpType.add)
                        ct_ps = pss.tile([128, 15], f32, name=f"ct_{tagn}", tag=f"ct{tagn}")
                        nc.tensor.matmul(ct_ps[:], lhsT=ones[:], rhs=pcnt[:], start=True, stop=True)
                        flag = sw.tile([128, 15], f32, name=f"fl_{tagn}", tag=f"fl{tagn}")
                        nc.vector.tensor_tensor(out=flag[:], in0=ct_ps[:], in1=kf[:].broadcast_to([128, 15]), op=AluOpType.is_ge)
                        fl2 = sw.tile([128, 15], f32, name=f"fl2_{tagn}", tag=f"fl2{tagn}")
                        nc.vector.tensor_scalar(out=fl2[:], in0=flag[:], scalar1=HUGE, scalar2=None, op0=AluOpType.mult)
                        sel = sw.tile([128, 15], f32, name=f"sel_{tagn}", tag=f"sel{tagn}")
                        nc.vector.tensor_tensor(out=sel[:], in0=pr[:], in1=fl2[:], op=AluOpType.subtract)
                        nl = sw.tile([128, 1], f32, name=f"nl_{tagn}", tag=f"nl{tagn}")
                        nc.vector.tensor_reduce(nl[:], sel[:], axis=AX.X, op=AluOpType.max)
                        nc.vector.tensor_tensor(out=lo[:], in0=lo[:], in1=nl[:], op=AluOpType.max)
                        t2 = sw.tile([128, 15], f32, name=f"t2_{tagn}", tag=f"t2{tagn}")
                        nc.vector.tensor_scalar(out=t2[:], in0=fl2[:], scalar1=-1.0, scalar2=HUGE, op0=AluOpType.mult, op1=AluOpType.add)
                        nc.vector.tensor_tensor(out=sel[:], in0=pr[:], in1=t2[:], op=AluOpType.add)
                        nh = sw.tile([128, 1], f32, name=f"nh_{tagn}", tag=f"nh{tagn}")
                        nc.vector.tensor_reduce(nh[:], sel[:], axis=AX.X, op=AluOpType.min)
                        nc.vector.tensor_tensor(out=hi[:], in0=hi[:], in1=nh[:], op=AluOpType.min)
                    return lo

                bstar = kselect(hb, kk_f, HB_HI, 7, "h")          # high-23 bits of thr

                # r1 = count(hb < B*), k2 = k - r1
                cmpb = sw.tile([128, AI], f32)
                nc.vector.tensor_tensor(out=cmpb[:], in0=hb[:], in1=bstar[:].broadcast_to([128, AI]), op=AluOpType.is_lt)
                r1p = sw.tile([128, 1], f32)
                nc.vector.tensor_reduce(r1p[:], cmpb[:], axis=AX.X, op=AluOpType.add)
                r1_ps = pss.tile([128, 1], f32)
                nc.tensor.matmul(r1_ps[:], lhsT=ones[:], rhs=r1p[:], start=True, stop=True)
                k2f = sw.tile([128, 1], f32)
                nc.vector.tensor_tensor(out=k2f[:], in0=kk_f[:], in1=r1_ps[:], op=AluOpType.subtract)
                r1 = sw.tile([128, 1], f32)
                nc.vector.tensor_copy(r1[:], r1_ps[:])

                # cand = lb where hb==B* else 256
                eqb = sw.tile([128, AI], f32)
                nc.vector.tensor_tensor(out=eqb[:], in0=hb[:], in1=bstar[:].broadcast_to([128, AI]), op=AluOpType.is_equal)
                cand = sw.tile([128, AI], f32)
                nc.vector.tensor_tensor(out=cand[:], in0=lb[:], in1=eqb[:], op=AluOpType.mult)
                inv2 = sw.tile([128, AI], f32)
                nc.vector.tensor_scalar(out=inv2[:], in0=eqb[:], scalar1=-256.0, scalar2=256.0, op0=AluOpType.mult, op1=AluOpType.add)
                nc.vector.tensor_tensor(out=cand[:], in0=cand[:], in1=inv2[:], op=AluOpType.add)

                lstar = kselect(cand, k2f, 257, 4, "l")           # low-8 bits of thr

                # keep = (hb < B*) | (cand < L*)   (disjoint)
                keep = sw.tile([128, AI], f32)
                cl = sw.tile([128, AI], f32)
                nc.vector.tensor_tensor(out=cl[:], in0=cand[:], in1=lstar[:].broadcast_to([128, AI]), op=AluOpType.is_lt)
                nc.vector.tensor_tensor(out=keep[:], in0=cmpb[:], in1=cl[:], op=AluOpType.add)

                # ---------- final loss ----------
                mk = sw.tile([128, AI], f32)
                nc.vector.tensor_tensor(out=mk[:], in0=keep[:], in1=mnat[:], op=AluOpType.mult)
                d2 = sw.tile([128, AI], f32)
                nc.vector.tensor_tensor(out=d2[:], in0=diff[:], in1=diff[:], op=AluOpType.mult)
                nc.vector.tensor_tensor(out=d2[:], in0=d2[:], in1=mk[:], op=AluOpType.mult)
                s2 = sw.tile([128, 2], f32)
                nc.vector.tensor_reduce(s2[:, 0:1], d2[:], axis=AX.X, op=AluOpType.add)
                nc.vector.tensor_reduce(s2[:, 1:2], mk[:], axis=AX.X, op=AluOpType.add)
                s2_ps = pss.tile([128, 2], f32)
                nc.tensor.matmul(s2_ps[:], lhsT=ones[:], rhs=s2[:], start=True, stop=True)
                s2a = sw.tile([128, 2], f32)
                nc.vector.tensor_copy(s2a[:], s2_ps[:])
                den = sw.tile([128, 1], f32)
                nc.vector.tensor_scalar(out=den[:], in0=s2a[:, 1:2], scalar1=1e-12, scalar2=None, op0=AluOpType.add)
                rden = sw.tile([128, 1], f32)
                nc.vector.reciprocal(rden[:], den[:])
                lb_t = sw.tile([128, 1], f32)
                nc.vector.tensor_tensor(out=lb_t[:], in0=s2a[:, 0:1], in1=rden[:], op=AluOpType.mult)
                nc.vector.tensor_scalar(out=lb_t[:], in0=lb_t[:], scalar1=0.125, scalar2=None, op0=AluOpType.mult)

                # global mean over batches: AllReduce(add) of loss_b/8 over 8 cores
                cc2i = dp.tile([1, 1], f32)
                cc2o = dp.tile([1, 1], f32)
                nc.sync.dma_start(cc2i[:], lb_t[0:1, 0:1])
                nc.gpsimd.collective_compute(
                    "AllReduce", AluOpType.add,
                    replica_groups=[[0, 1, 2, 3, 4, 5, 6, 7]],
                    ins=[cc2i[:]], outs=[cc2o[:]])
                lossg = sw.tile([1, 1], f32)
                nc.sync.dma_start(lossg[:], cc2o[:])

                # out = exp(-alpha) * loss + alpha
                ea = sw.tile([1, 1], f32)
                nc.scalar.activation(ea[:], alph[:], AF.Exp, scale=-1.0)
                ov = sw.tile([1, 1], f32)
                nc.vector.tensor_tensor(out=ov[:], in0=ea[:], in1=lossg[:], op=AluOpType.mult)
                nc.vector.tensor_tensor(out=ov[:], in0=ov[:], in1=alph[:], op=AluOpType.add)
                nc.sync.dma_start(out_d[:], ov[:])

                # debug row: n_ip, n_it, n_sel, k, B*, L*, r1, loss_b*8... (per-partition col dump)
                dbgt = sw.tile([128, 8], f32)
                nc.vector.tensor_copy(dbgt[:, 0:1], c2a[:, 0:1])
                nc.vector.tensor_copy(dbgt[:, 1:2], c2a[:, 1:2])
                nc.vector.tensor_copy(dbgt[:, 2:3], nsa[:])
                nc.vector.tensor_copy(dbgt[:, 3:4], kk_f[:])
                nc.vector.tensor_copy(dbgt[:, 4:5], bstar[:])
                nc.vector.tensor_copy(dbgt[:, 5:6], lstar[:])
                nc.vector.tensor_copy(dbgt[:, 6:7], r1[:])
                nc.vector.tensor_copy(dbgt[:, 7:8], lb_t[:])
                nc.sync.dma_start(dbg_d[:], dbgt[:])

    return nc


# --------------------------------------------------------------------------
# host wrapper
# --------------------------------------------------------------------------
_NC_CACHE = {}


def _get_nc():
    if 'nc' not in _NC_CACHE:
        _NC_CACHE['nc'] = build_nc()
    return _NC_CACHE['nc']


def _marshal(prediction_tensor, target_tensor, mask, alpha):
    pred = np.asarray(prediction_tensor, np.float32)
    tgt = np.asarray(target_tensor, np.float32)
    msk = np.asarray(mask, np.float32)
    al = np.asarray(alpha, np.float32).reshape(1, 1)

    in_maps = []
    for c in range(N_CORES):
        b, h = c // 2, c % 2
        p = np.empty((NI, 3), np.float32)
        p[:N] = pred[b]
        p[N:] = pred[b, 0]
        t = np.full((NI, 3), PADV, np.float32)
        t[:N] = tgt[b]
        th = np.full((NJ, 3), PADV, np.float32)
        th[:MH] = tgt[b, h * MH:(h + 1) * MH]
        m = np.zeros(NI, np.float32)
        m[:N] = msk[b]
        in_maps.append({
            'pred_pm': np.ascontiguousarray(p.reshape(128, AI * 3)),
            'pred_nat': np.ascontiguousarray(
                p.reshape(AI, 128, 3).transpose(1, 0, 2).reshape(128, AI * 3)),
            'tgt_nat': np.ascontiguousarray(
                t.reshape(AI, 128, 3).transpose(1, 0, 2).reshape(128, AI * 3)),
            'tgt_half_pm': np.ascontiguousarray(th.reshape(128, AJ * 3)),
            'mask_nat': np.ascontiguousarray(m.reshape(AI, 128).T),
            'valid_nat': np.ascontiguousarray(
                (np.arange(NI) < N).astype(np.float32).reshape(AI, 128).T),
            'alpha_in': al,
        })
    return in_maps


def run_cores(prediction_tensor, target_tensor, mask, alpha, **rb_kwargs):
    nc = _get_nc()
    in_maps = _marshal(prediction_tensor, target_tensor, mask, alpha)
    return run_bass_kernel_spmd(nc, in_maps, core_ids=list(range(N_CORES)), **rb_kwargs)


def kernel(prediction_tensor, target_tensor, mask, alpha):
    res = run_cores(prediction_tensor, target_tensor, mask, alpha)
    return res.results[0]['out'].reshape(1).astype(np.float32)
